# revision 145
# baseline (speedup 1.0000x reference)
"""MLA (DeepSeek-style multi-head latent attention) kernel for Trainium2.

Problem: nn_MultiHeadAttention_28243704939173
  B=2, S=2048, D=2048, H=16, KV_RANK=512, NOPE=128, ROPE=64, V_HD=128.

Sharding (8 NeuronCores): DP=2 over batch x TP=4 over heads (4 heads per
core). The kv-latent projection is sharded over the 4 TP ranks (each rank
projects S/4 contiguous token blocks, fed via the `xkv` input) and the
normalized kv^T shards are exchanged with an in-kernel AllGather over
replica groups [[0..3],[4..7]], overlapped with the q projection. Each
core emits its heads' partial wo projection transposed ([D, S]); the host
sums the 4 TP partials per batch element and adds wo_b.

Structure:
  * Per-head K/V materialization: k_eff_h^T = (wb_h norm) @ kv^T and
    v_eff_h = kv @ (wv_h norm)^T (both 128-d) are small GEMMs; scores then
    contract over 192 dims and PV over 128, instead of both in the 512-d
    latent space (~2.4x less PE work than the absorbed-latent form).
  * Everything runs in bf16 (full PE rate, half the DMA/SBUF, 2x DVE on
    packed copies); PSUM accumulation is fp32. Final rel err ~3e-3 vs the
    2e-2 gate.
  * q-nope is projected DIRECTLY TRANSPOSED (weight chunks stationary, x^T
    moving -> [d, tok] PSUM), so qnt needs no PE transposes; only the rope
    cols (q-rope + k-rope, 320 wide) go token-major for the DVE rope chain
    and get per-block PE transposes.
  * Phase 3 computes scores TRANSPOSED (keys on partitions): the Act exp
    output in SBUF is P^T and feeds the PV matmul directly -- no P
    transposes or PSUM->SBUF P copies. Off-diagonal strip pairs share one
    2-bank PSUM tile / one exp instruction; the causal triangle of the
    diagonal block is a post-exp 0/1 bf16 multiply. The softmax denominator
    is built OFF the PE: a DVE add-tree over the exp strips, a gpsimd
    partition_all_reduce (which also broadcasts), then reciprocal+multiply
    on the oT store. G0's tiny groups interleave with G1's, and wo chunks
    ride inside the attention loop as PE filler (sharing the po PSUM ring),
    so only the last 512-query group's wo runs after the attention drain.
  * The cost model serializes all DMA transfers on one engine pool, so the
    startup issues every front transfer on ONE queue in exact PE-consumption
    order (wkv chunk0, xkb0, remaining wkv, xkb1-3, rope-col weights, rope
    tables, x for q-group 0, nope weights); junk matmuls on a memset tile
    warm the PE pstate inside the initial DMA shadow, and pass-A blocks 0/1
    interleave at half-block granularity to ride the arrival curve. The kv^T
    AllGather (~80us hidden under pass B) feeds phase K, whose psums share
    the phase-3 rings inside one PSUM scope (no boundary stall).
"""
import numpy as np
from contextlib import ExitStack

import ml_dtypes

import concourse.bacc as bacc
import concourse.bass_isa as bass_isa
import concourse.mybir as mybir
import concourse.tile as tile
from concourse import bass_utils

F32 = mybir.dt.float32
BF16 = mybir.dt.bfloat16
AF = mybir.ActivationFunctionType
ALU = mybir.AluOpType

B, S, D = 2, 2048, 2048
H = 16
KV = 512
NOPE, ROPE = 128, 64
QK_HD = NOPE + ROPE
V_HD = 128
SCALE = float(QK_HD) ** -0.5
EPS = 1.1920929e-07
NEG = -1.0e5  # mask addend; NEG*SCALE ~ -7220 -> exp underflows to exactly 0
HL = 4        # local heads per core (TP degree 4)
TP = 4
N_CORES = 8
KD = D // 128   # contraction chunks over the model dim
QW = 832        # fused q-projection width: 4*128 nope + 4*64 qrope + 64 krope


def to_bf16(a: np.ndarray) -> np.ndarray:
    return np.ascontiguousarray(a).astype(ml_dtypes.bfloat16)


def build(s_len: int, q_bias: bool, kv_bias: bool):
    NB = s_len // 128

    nc = bacc.Bacc("TRN2", target_bir_lowering=False, debug=False)

    xt = nc.dram_tensor("xt", [NB, 128, D], BF16, kind="ExternalInput")
    xkv = nc.dram_tensor("xkv", [NB // 4, 128, D], BF16, kind="ExternalInput")
    wqn = nc.dram_tensor("wqn", [128, KD * 512], BF16, kind="ExternalInput")
    wqr = nc.dram_tensor("wqr", [128, KD * 320], BF16, kind="ExternalInput")
    wkv = nc.dram_tensor("wkv", [128, KD * 512], BF16, kind="ExternalInput")
    wbmt = nc.dram_tensor("wbmt", [128, HL * 512], BF16, kind="ExternalInput")
    wvt = nc.dram_tensor("wvt", [128, HL * 512], BF16, kind="ExternalInput")
    wot = nc.dram_tensor("wot", [128, HL * D], BF16, kind="ExternalInput")
    cosq = nc.dram_tensor("cosq", [128, NB * 64], BF16, kind="ExternalInput")
    sinq = nc.dram_tensor("sinq", [128, NB * 64], BF16, kind="ExternalInput")
    dmask = nc.dram_tensor("dmask", [128, 128], BF16, kind="ExternalInput")
    identb = nc.dram_tensor("identb", [128, 128], BF16, kind="ExternalInput")
    if q_bias:
        qb = nc.dram_tensor("qb", [1, QW], BF16, kind="ExternalInput")
    if kv_bias:
        kvb = nc.dram_tensor("kvb", [1, 512], BF16, kind="ExternalInput")
    outT = nc.dram_tensor("outT", [128, D // 128, s_len], BF16,
                          kind="ExternalOutput")

    with tile.TileContext(nc) as tc, ExitStack() as ctx:
        # ---------------- persistent tensors (whole kernel) -----------------
        pe = ctx.enter_context(tc.tile_pool(name="pe", bufs=1))
        identb_sb = pe.tile([128, 128], BF16, tag="identb_sb")
        dmask_sb = pe.tile([128, 128], BF16, tag="dmask_sb")
        qnt = pe.tile([128, HL * s_len], BF16, tag="qnt")       # [d, (h,q)]
        qpet = [pe.tile([128, s_len], BF16, tag=f"qpet{pp}", name=f"qpet{pp}")
                for pp in range(2)]                             # [(2h,r), q]
        kpet = pe.tile([128, s_len], BF16, tag="kpet")          # [r x2, t]
        kvt = pe.tile([128, 4 * s_len], BF16, tag="kvt")        # [c, (cc,t)]
        wbmt_sb = pe.tile([128, HL * 512], BF16, tag="wbmt_sb")
        wvt_sb = pe.tile([128, HL * 512], BF16, tag="wvt_sb")

        # ========== Phase 12: fused kv-latent + q/k-rope projections ========
        with tc.tile_pool(name="p12w", bufs=1) as p12w, \
                tc.tile_pool(name="p12", bufs=3) as p12, \
                tc.tile_pool(name="p12s", bufs=6) as p12s, \
                tc.tile_pool(name="pst", bufs=2, space="PSUM") as pst, \
                tc.tile_pool(name="psq2", bufs=2, space="PSUM") as psq2:
            # Startup DMAs: the cost model serializes all transfers on one
            # DMA-engine resource, so the GLOBAL arrival order must track the
            # PE's consumption order: wkv chunk0 + xkb0 first (pass A block
            # 0), the remaining wkv chunks while block 0 runs, then xkb1-3,
            # then wq groups + xg0 blocks for pass B, misc last. Orders are
            # interleaved across the SP/Act queues (shared HWDGE round-robin).
            NLOC = NB // 4
            xkb = [p12.tile([128, 2048], BF16, tag="xkb", name=f"xkb{ss}",
                            bufs=4) for ss in range(NLOC)]
            wkv_splits = [(0, 1), (1, 3), (4, 4), (8, 4), (12, 4)]
            wkvg = {}
            wkvc = [None] * KD
            for k0, nk_ in wkv_splits:
                wt = p12w.tile([128, 512 * nk_], BF16, tag=f"wkvg{k0}",
                               name=f"wkvg{k0}")
                wkvg[k0] = (wt, nk_)
                for kk in range(nk_):
                    wkvc[k0 + kk] = wt[:, 512 * kk:512 * (kk + 1)]

            def dma_wkv(eng, k0):
                wt, nk_ = wkvg[k0]
                eng.dma_start(wt[:], wkv.ap()[:, 512 * k0:512 * (k0 + nk_)])

            cosq_sb = p12w.tile([128, NB * 64], BF16, tag="cosq_sb")
            sinq_sb = p12w.tile([128, NB * 64], BF16, tag="sinq_sb")
            # q weights split by column type: rope cols are needed first
            # (group-0 ropes run right after pass A), nope cols ~8us later
            wqng = [p12w.tile([128, 4 * 512], BF16, tag="wqng",
                              name=f"wqng{i}", bufs=4) for i in range(4)]
            wqrg = [p12w.tile([128, 4 * 320], BF16, tag="wqrg",
                              name=f"wqrg{i}", bufs=4) for i in range(4)]
            wqnc = [wqng[k // 4][:, 512 * (k % 4):512 * (k % 4 + 1)]
                    for k in range(KD)]
            wqrc = [wqrg[k // 4][:, 320 * (k % 4):320 * (k % 4 + 1)]
                    for k in range(KD)]

            xg0 = p12.tile([128, 4 * 2048], BF16, tag="xg", bufs=3, name="xg0")
            xg0v = xg0[:].rearrange("p (b k c) -> p b k c", b=4, k=KD)

            # PE warm-up: junk matmuls on a memset tile during the initial
            # DMA wait, so the pstate ramp (HAM on real hw) finishes in the
            # shadow and pass A runs at full rate from its first matmul.
            jt = p12w.tile([128, 320], BF16, tag="jt")
            nc.vector.memset(jt[:], 0.0)

            def emit_junk(n):
                for _ in range(n):
                    pj = psq2.tile([128, 320], F32, tag="pq2")
                    nc.tensor.matmul(pj[:], jt[:, 0:128], jt[:],
                                     start=True, stop=True,
                                     skip_group_check=True)

            emit_junk(14)

            # -- the ordered startup sequence: ONE queue (SP), strict demand
            # order, so nothing jumps ahead in the serialized transfer order.
            # (identb rides the Pool SWDGE: 128x128, negligible.)
            dma_wkv(nc.sync, 0)
            nc.sync.dma_start(xkb[0][:], xkv.ap()[0][:, :])
            dma_wkv(nc.sync, 1)
            dma_wkv(nc.sync, 4)
            nc.sync.dma_start(xkb[1][:], xkv.ap()[1][:, :])
            dma_wkv(nc.sync, 8)
            dma_wkv(nc.sync, 12)
            nc.sync.dma_start(xkb[2][:], xkv.ap()[2][:, :])
            nc.sync.dma_start(wqrg[0][:], wqr.ap()[:, 0:1280])
            nc.sync.dma_start(xkb[3][:], xkv.ap()[3][:, :])
            for i in range(1, 4):
                nc.sync.dma_start(wqrg[i][:],
                                  wqr.ap()[:, 1280 * i:1280 * (i + 1)])
            # rope tables: group-0's slice first, the rest off the front
            nc.sync.dma_start(cosq_sb[:, 0:256], cosq.ap()[:, 0:256])
            nc.sync.dma_start(sinq_sb[:, 0:256], sinq.ap()[:, 0:256])
            nc.sync.dma_start(xg0v[:, 0, :, :], xt.ap()[0][:, :])
            for b in range(1, 4):
                nc.sync.dma_start(xg0v[:, b, :, :], xt.ap()[b][:, :])
            for i in range(4):
                nc.sync.dma_start(wqng[i][:],
                                  wqn.ap()[:, 2048 * i:2048 * (i + 1)])
            nc.sync.dma_start(cosq_sb[:, 256:NB * 64], cosq.ap()[:, 256:NB * 64])
            nc.sync.dma_start(sinq_sb[:, 256:NB * 64], sinq.ap()[:, 256:NB * 64])
            nc.sync.dma_start(dmask_sb[:], dmask.ap()[:])
            nc.gpsimd.dma_start(identb_sb[:], identb.ap()[:])
            nc.sync.dma_start(wbmt_sb[:], wbmt.ap()[:])
            nc.sync.dma_start(wvt_sb[:], wvt.ap()[:])
            if q_bias or kv_bias:
                ones1 = p12w.tile([1, 128], BF16, tag="ones1")
                nc.vector.memset(ones1[:], 1.0)
            if q_bias:
                qb_sb = p12w.tile([1, QW], BF16, tag="qb_sb")
                nc.scalar.dma_start(qb_sb[:], qb.ap()[:])
                ones512 = p12w.tile([1, 512], BF16, tag="ones512")
                nc.vector.memset(ones512[:], 1.0)
            if kv_bias:
                kvb_sb = p12w.tile([1, 512], BF16, tag="kvb_sb")
                nc.scalar.dma_start(kvb_sb[:], kvb.ap()[:])

            qnt4 = qnt[:].rearrange("p (h n c) -> p h n c", h=HL, n=NB)
            kvt4 = kvt[:].rearrange("p (cc n c) -> p cc n c", cc=4, n=NB)

            # ---- pass A: kv latent for this rank's NB/4 blocks (sharded) ----
            kvsh = p12w.tile([128, 4 * 128 * NLOC], BF16, tag="kvsh")
            kvsh4 = kvsh[:].rearrange("p (cc n c) -> p cc n c", cc=4, n=NLOC)

            def emit_tr_kv(st):
                ss, kv_bf = st
                ptc = pst.tile([128, 512], BF16, tag="ptc")
                for cc in range(4):
                    nc.tensor.transpose(ptc[:, 128 * cc:128 * (cc + 1)],
                                        kv_bf[:, 128 * cc:128 * (cc + 1)],
                                        identb_sb[:])
                nc.vector.tensor_copy(
                    kvsh4[:, :, ss, :],
                    ptc[:].rearrange("p (cc c) -> p cc c", cc=4))

            prevkv = None

            pkvs = {}

            def emit_passA_mm(ss, ka, kb):
                if ka == 0:
                    pkvs[ss] = pskv.tile([128, 512], F32, tag="pkv",
                                         name=f"pkv{ss}")
                pkv = pkvs[ss]
                for k in range(ka, kb):
                    # filler so the warm PE never idles (or resets its
                    # pstate) while the weight-chunk stream catches up
                    if ss == 0 and k in (2, 4, 6, 10, 13):
                        emit_junk(2)
                    lhs = xkb[ss][:, 128 * k:128 * (k + 1)]
                    nc.tensor.matmul(pkv[:], lhs, wkvc[k][:],
                                     start=(k == 0),
                                     stop=(k == KD - 1 and not kv_bias),
                                     skip_group_check=True)
                if kb == KD and kv_bias:
                    nc.tensor.matmul(pkv[:], ones1[:], kvb_sb[:],
                                     start=False, stop=True,
                                     skip_group_check=True)

            def emit_passA(ss):
                nonlocal prevkv
                pkv = pkvs[ss]
                if prevkv is not None:
                    emit_tr_kv(prevkv)
                kvtile = p12.tile([128, 512], F32, tag="kvtile")
                nc.scalar.copy(kvtile[:], pkv[:])
                sq = p12.tile([128, 512], F32, tag="sq", bufs=3)
                msq = p12s.tile([128, 1], F32, tag="msq")
                nc.scalar.activation(sq[:], kvtile[:], AF.Square, bias=0.0,
                                     scale=1.0, accum_out=msq[:])
                ms2 = p12s.tile([128, 1], F32, tag="ms2")
                nc.vector.tensor_scalar(ms2[:], msq[:], 1.0 / KV, EPS, ALU.mult,
                                        ALU.add)
                srt = p12s.tile([128, 1], F32, tag="srt")
                nc.scalar.sqrt(srt[:], ms2[:])
                rrt = p12s.tile([128, 1], F32, tag="rrt")
                nc.vector.reciprocal(rrt[:], srt[:])
                kv_bf = p12.tile([128, 512], BF16, tag="kv_bf", bufs=4)
                nc.vector.tensor_scalar(kv_bf[:], kvtile[:], rrt[:], None,
                                        ALU.mult)
                # krope: bf16 rope chain on this rank's block (sharded)
                prevkv = (ss, kv_bf)

            def emit_collective(ccd, in_b):
                out_b = ccd.tile([TP, 128, 4 * 128 * NLOC], BF16,
                                 name="kv_ag_out")
                nc.gpsimd.dma_start(in_b[:], kvsh[:])
                nc.gpsimd.collective_compute(
                    "AllGather", ALU.bypass,
                    replica_groups=[[0, 1, 2, 3], [4, 5, 6, 7]],
                    ins=[in_b.opt()], outs=[out_b.opt()])
                for r in range(TP):
                    nc.gpsimd.dma_start(
                        kvt4[:, :, NLOC * r:NLOC * (r + 1), :],
                        out_b[r].rearrange("p (cc n c) -> p cc n c",
                                           cc=4, n=NLOC))

            prev = None

            def emit_tr(st):
                """PE transposes for the rope cols of block st (lagged one)."""
                s, qro_bf = st
                ptp = pst.tile([128, 512], BF16, tag="ptc")
                for pp in range(2):
                    nc.tensor.transpose(ptp[:, 128 * pp:128 * (pp + 1)],
                                        qro_bf[:, 128 * pp:128 * (pp + 1)],
                                        identb_sb[:])
                    nc.vector.tensor_copy(qpet[pp][:, 128 * s:128 * (s + 1)],
                                          ptp[:, 128 * pp:128 * (pp + 1)])
                # kpeT duplicated on both partition halves (the rope matmul
                # pairs it with either half of qpet); DVE cannot cross
                # partitions, so transpose twice with an explicit placement.
                nc.tensor.transpose(ptp[0:64, 256:384], qro_bf[:, 256:320],
                                    identb_sb[:], tile_position=(0, 0))
                nc.tensor.transpose(ptp[64:128, 256:384], qro_bf[:, 256:320],
                                    identb_sb[:], tile_position=(0, 64))
                nc.vector.tensor_copy(kpet[:, 128 * s:128 * (s + 1)],
                                      ptp[:, 256:384])

            def emit_rope(s, xg4):
                """Rope q-cols (+krope) for one 128-token block, token-major."""
                nonlocal prev
                b = s % 4
                pq2 = psq2.tile([128, 320], F32, tag="pq2")
                for k in range(KD):
                    nc.tensor.matmul(pq2[:], xg4[:, b, k, :], wqrc[k][:],
                                     start=(k == 0),
                                     stop=(k == KD - 1 and not q_bias),
                                     skip_group_check=True)
                if q_bias:
                    nc.tensor.matmul(pq2[:], ones1[:], qb_sb[:, 512:832],
                                     start=False, stop=True,
                                     skip_group_check=True)
                if prev is not None:
                    emit_tr(prev)
                # whole rope chain in bf16: 2x DVE modes, no final copy (qr
                # stays alive for the lagged transposes, hence bufs=3)
                qr = p12.tile([128, 320], BF16, tag="qr", bufs=4)
                nc.scalar.copy(qr[:], pq2[:])
                qsw = p12.tile([128, 320], BF16, tag="qsw")
                a3 = qr[:].rearrange("p (i two) -> p i two", two=2)
                w3 = qsw[:].rearrange("p (i two) -> p i two", two=2)
                nc.vector.tensor_copy(w3[:, :, 0:1], a3[:, :, 1:2])
                nc.vector.tensor_copy(w3[:, :, 1:2], a3[:, :, 0:1])
                cs = cosq_sb[:, 64 * s:64 * (s + 1)]
                sn = sinq_sb[:, 64 * s:64 * (s + 1)]
                for r in range(5):
                    rsl = qr[:, 64 * r:64 * (r + 1)]
                    ssl = qsw[:, 64 * r:64 * (r + 1)]
                    nc.vector.tensor_mul(rsl, rsl, cs)
                    nc.vector.tensor_mul(ssl, ssl, sn)
                    nc.vector.tensor_add(rsl, rsl, ssl)
                prev = (s, qr)

            def alloc_xg(g):
                """Prefetch the next group's x blocks (ring of 3)."""
                xg = p12.tile([128, 4 * 2048], BF16, tag="xg", bufs=3,
                              name=f"xg{g}")
                xg4 = xg[:].rearrange("p (b k c) -> p b k c", b=4, k=KD)
                for b in range(4):
                    nc.sync.dma_start(xg4[:, b, :, :],
                                      xt.ap()[4 * g + b][:, :])
                return xg4

            def emit_nope(g, xg4, psq1, with_rope):
                """q-nope for one 512-token group: directly transposed
                (weights stationary, x^T moving -> [d, tok] PSUM, no PE
                transposes). Group 0 goes block-by-block with a k-outer loop
                so its x/weight demand is paced with the startup DMA stream;
                later groups (data resident) use group-wide N=512 matmuls."""
                pqts = [psq1.tile([128, 512], F32, tag="pqt",
                                  name=f"pqt{g}_{h}") for h in range(HL)]
                if g == 0:
                    for b in range(4):
                        for k in range(KD):
                            for h in range(HL):
                                nc.tensor.matmul(
                                    pqts[h][:, 128 * b:128 * (b + 1)],
                                    wqnc[k][:, 128 * h:128 * (h + 1)],
                                    xg4[:, b, k, :],
                                    start=(k == 0),
                                    stop=(k == KD - 1 and not q_bias),
                                    skip_group_check=True)
                        if q_bias:
                            for h in range(HL):
                                nc.tensor.matmul(
                                    pqts[h][:, 128 * b:128 * (b + 1)],
                                    qb_sb[:, 128 * h:128 * (h + 1)], ones1[:],
                                    start=False, stop=True,
                                    skip_group_check=True)
                else:
                    for h in range(HL):
                        for k in range(KD):
                            nc.tensor.matmul(
                                pqts[h][:],
                                wqnc[k][:, 128 * h:128 * (h + 1)],
                                xg4[:, :, k, :],
                                start=(k == 0),
                                stop=(k == KD - 1 and not q_bias),
                                skip_group_check=True)
                        if q_bias:
                            nc.tensor.matmul(
                                pqts[h][:],
                                qb_sb[:, 128 * h:128 * (h + 1)], ones512[:],
                                start=False, stop=True, skip_group_check=True)
                        if with_rope:
                            emit_rope(4 * g + h, xg4)
                        # copy as soon as this head's psum completes, so the
                        # next group's pqt ring slot frees ~10us earlier
                        nc.scalar.copy(qnt4[:, h, 4 * g:4 * (g + 1), :]
                                       .rearrange("p n c -> p (n c)"),
                                       pqts[h][:])
                if g == 0:
                    if with_rope:
                        for b in range(4):
                            emit_rope(4 * g + b, xg4)
                    for h in range(HL):
                        nc.scalar.copy(qnt4[:, h, 4 * g:4 * (g + 1), :]
                                       .rearrange("p n c -> p (n c)"),
                                       pqts[h][:])

            # order: pass A; collective; group-0 ropes (fed by the early
            # rope-col weights); group-0 nope; groups 1-3 (nope+rope)
            with tc.tile_pool(name="ccd", bufs=1, space="DRAM") as ccd:
                in_b = ccd.tile([128, 4 * 128 * NLOC], BF16,
                                name="kv_ag_in")
                with tc.tile_pool(name="pskv", bufs=2, space="PSUM") as pskv:
                    # blocks 0/1 interleaved at half-block granularity:
                    # block 1 (fed by the early xkb1 transfer) fills block
                    # 0's weight-arrival stalls
                    emit_passA_mm(0, 0, 8)
                    emit_passA_mm(1, 0, 8)
                    emit_passA_mm(0, 8, KD)
                    emit_passA(0)
                    emit_passA_mm(1, 8, KD)
                    emit_passA(1)
                    for i in range(2, NLOC):
                        emit_passA_mm(i, 0, KD)
                        emit_passA(i)
                    emit_tr_kv(prevkv)
                emit_collective(ccd, in_b)
                for b in range(4):
                    emit_rope(b, xg0v)
                with tc.tile_pool(name="psq1", bufs=4, space="PSUM") as psq1:
                    nxt = alloc_xg(1)
                    emit_nope(0, xg0v, psq1, with_rope=False)
                    for g in range(1, NB // 4):
                        cur = nxt
                        if g < NB // 4 - 1:
                            nxt = alloc_xg(g + 1)
                        emit_nope(g, cur, psq1, with_rope=True)
                    emit_tr(prev)

        # ---------------- persistent tensors (post-P12) ---------------------
        pl = ctx.enter_context(tc.tile_pool(name="pl", bufs=1))
        keff = pl.tile([128, HL * s_len], BF16, tag="keff")   # [d, (h,t)]
        veff = pl.tile([128, HL * s_len], BF16, tag="veff")   # [t, (h,j*d)]
        ot_bf = pl.tile([128, HL * s_len], BF16, tag="ot_bf")  # [d, (h,q)]
        wot_sb = pl.tile([128, HL * D], BF16, tag="wot_sb")
        nc.scalar.dma_start(wot_sb[:], wot.ap()[:])

        # ========== Phase K: materialize per-head K/V =======================
        NG = s_len // 512

        # ========== Phases K+3 (one PSUM scope, no boundary stall) =========
        # Scores are computed transposed (keys on partitions, queries on the
        # free dim), so the Act exp output in SBUF IS P^T and feeds the PV
        # matmul directly -- no transposes, no PSUM->SBUF P copies. The
        # softmax denominator is built OFF the PE: a DVE add-tree over the
        # exp strips followed by a gpsimd partition_all_reduce (which also
        # broadcasts), then a single DVE divide on the oT store. Off-diagonal
        # strips are paired into 2-bank PSUM tiles so one exp instruction
        # covers both. The causal mask inside the diagonal 128x128 block is a
        # post-exp multiply by a 0/1 bf16 triangle (exp(x-1e5*m) == exp(x)*t).
        with tc.tile_pool(name="expp", bufs=1) as expp, \
                tc.tile_pool(name="accp", bufs=4) as accp, \
                tc.tile_pool(name="rzp", bufs=4) as rzp, \
                tc.tile_pool(name="pssp", bufs=2, space="PSUM") as pssp, \
                tc.tile_pool(name="pssd", bufs=2, space="PSUM") as pssd, \
                tc.tile_pool(name="ps3o", bufs=2, space="PSUM") as ps3o:
            # ---- phase K: per-head K/V materialization (psums share the
            # pssd ring so the K->3 transition reuses warm banks) ----
            for h in range(HL):
                for tg in range(NG):
                    pk = pssd.tile([128, 512], F32, tag="psd")
                    for cc in range(4):
                        nc.tensor.matmul(
                            pk[:],
                            wbmt_sb[:, 512 * h + 128 * cc:512 * h + 128 * (cc + 1)],
                            kvt[:, s_len * cc + 512 * tg:s_len * cc + 512 * (tg + 1)],
                            start=(cc == 0), stop=(cc == 3))
                    nc.scalar.copy(
                        keff[:, s_len * h + 512 * tg:s_len * h + 512 * (tg + 1)],
                        pk[:])
            # veff: one [t, (h,d)] psum per 128-token block -- N=512 matmuls
            # across all 4 heads at once (4x fewer instructions than per-head)
            wvt4 = wvt_sb[:].rearrange("p (h c) -> p h c", h=HL)
            veff3 = veff[:].rearrange("p (h t) -> p h t", h=HL)
            for j in range(NB):
                pv = ps3o.tile([128, 512], F32, tag="po")
                for cc in range(4):
                    nc.tensor.matmul(
                        pv[:],
                        kvt[:, s_len * cc + 128 * j:s_len * cc + 128 * (j + 1)],
                        wvt4[:, :, 128 * cc:128 * (cc + 1)],
                        start=(cc == 0), stop=(cc == 3),
                        skip_group_check=True)
                nc.vector.tensor_copy(
                    veff3[:, :, 128 * j:128 * (j + 1)],
                    pv[:].rearrange("p (h c) -> p h c", h=HL))

            def emit_scores(h, G):
                """Transposed scores + exp; off-diag strips paired per PSUM."""
                strips = []
                hb = 64 * (h % 2)
                q0 = s_len * h + 512 * G
                k0 = s_len * h

                def score_mms(dst, j, c0):
                    nc.tensor.matmul(
                        dst,
                        keff[:, k0 + 128 * j:k0 + 128 * (j + 1)],
                        qnt[:, q0 + 128 * c0:q0 + 512],
                        start=True, stop=False, skip_group_check=True)
                    nc.tensor.matmul(
                        dst,
                        kpet[hb:hb + 64, 128 * j:128 * (j + 1)],
                        qpet[h // 2][hb:hb + 64,
                                     512 * G + 128 * c0:512 * (G + 1)],
                        start=False, stop=True, skip_group_check=True)

                for jp in range(2 * G):
                    ps = pssp.tile([128, 1024], F32, tag="psp")
                    for u in range(2):
                        score_mms(ps[:, 512 * u:512 * (u + 1)], 2 * jp + u, 0)
                    et = expp.tile([128, 1024], BF16, tag="etp", bufs=15)
                    nc.scalar.activation(et[:], ps[:], AF.Exp, bias=0.0,
                                         scale=SCALE)
                    strips.append(("pair", jp, et))
                for ii in range(4):
                    j = 4 * G + ii
                    ps = pssd.tile([128, 512], F32, tag="psd")
                    score_mms(ps[:, 128 * ii:512], j, ii)
                    et = expp.tile([128, 512], BF16, tag="etd", bufs=11)
                    nc.scalar.activation(et[:, 128 * ii:512], ps[:, 128 * ii:512],
                                         AF.Exp, bias=0.0, scale=SCALE)
                    # zero the masked (t > q) triangle of the diagonal block
                    nc.vector.tensor_mul(et[:, 128 * ii:128 * (ii + 1)],
                                         et[:, 128 * ii:128 * (ii + 1)],
                                         dmask_sb[:])
                    strips.append(("diag", ii, et))
                return strips

            def emit_pv(h, G, strips):
                """PV accumulation + off-PE softmax denominator, /Z on store."""
                po = ps3o.tile([128, 512], F32, tag="po")
                acc = accp.tile([128, 512], BF16, tag="acc")
                last = 4 * G + 3
                k0 = s_len * h
                nacc = 0
                for kind, idx, et in strips:
                    if kind == "pair":
                        for u in range(2):
                            j = 2 * idx + u
                            nc.tensor.matmul(
                                po[:],
                                veff[:, k0 + 128 * j:k0 + 128 * (j + 1)],
                                et[:, 512 * u:512 * (u + 1)],
                                start=(j == 0), stop=(j == last),
                                skip_group_check=True)
                        if idx == 0:
                            nc.vector.tensor_add(acc[:], et[:, 0:512],
                                                 et[:, 512:1024])
                        else:
                            nc.vector.tensor_add(acc[:], acc[:], et[:, 0:512])
                            nc.vector.tensor_add(acc[:], acc[:], et[:, 512:1024])
                        nacc += 2
                    else:
                        ii = idx
                        j = 4 * G + ii
                        nc.tensor.matmul(
                            po[:, 128 * ii:512],
                            veff[:, k0 + 128 * j:k0 + 128 * (j + 1)],
                            et[:, 128 * ii:512],
                            start=(j == 0), stop=(j == last),
                            skip_group_check=True)
                        if nacc == 0:
                            nc.vector.tensor_copy(acc[:], et[:, 0:512])
                        else:
                            nc.vector.tensor_add(acc[:, 128 * ii:512],
                                                 acc[:, 128 * ii:512],
                                                 et[:, 128 * ii:512])
                        nacc += 1
                zr = rzp.tile([128, 512], F32, tag="zr")
                nc.gpsimd.partition_all_reduce(zr[:], acc[:], 128,
                                               bass_isa.ReduceOp.add)
                rz = rzp.tile([128, 512], F32, tag="rz")
                nc.vector.reciprocal(rz[:], zr[:])
                nc.vector.tensor_mul(
                    ot_bf[:, s_len * h + 512 * G:s_len * h + 512 * (G + 1)],
                    po[:], rz[:])

            # ---- phase 4 chunks (wo projection), interleaved into the
            # attention loop as PE filler: wo(G-1, ng) rides between the
            # score groups of G, sharing the "po" PSUM ring ----
            def emit_wo_chunk(G, ng, p4, tail=False):
                last_chunk = (G == NG - 1 and ng == D // 512 - 1)
                osb = p4.tile([128, 2048], BF16, tag="osb")
                for nn in range(4):
                    n = 4 * ng + nn
                    # after attention drains, the diag-score ring is idle --
                    # alternating psums halves the ring-handoff waits
                    pool_ = pssd if tail and nn % 2 else ps3o
                    pw = pool_.tile([128, 512], F32,
                                    tag="psd" if tail and nn % 2 else "po")
                    for h in range(HL):
                        nc.tensor.matmul(
                            pw[:],
                            wot_sb[:, D * h + 128 * n:D * h + 128 * (n + 1)],
                            ot_bf[:, s_len * h + 512 * G:s_len * h + 512 * (G + 1)],
                            start=(h == 0), stop=(h == HL - 1))
                    nc.scalar.copy(osb[:, 512 * nn:512 * (nn + 1)], pw[:])
                    if last_chunk:
                        eng = nc.sync if nn % 2 == 0 else nc.scalar
                        eng.dma_start(
                            outT.ap()[:, 4 * ng + nn, 512 * G:512 * (G + 1)],
                            osb[:, 512 * nn:512 * (nn + 1)])
                if not last_chunk:
                    nc.sync.dma_start(
                        outT.ap()[:, 4 * ng:4 * (ng + 1),
                                  512 * G:512 * (G + 1)],
                        osb[:].rearrange("p (n q) -> p n q", n=4))

            # G0's tiny groups interleave with G1's so the exp latency of the
            # short strips hides behind bigger PE work; wo chunks ride along
            # once their group's last divide is in flight.
            work = [(h // 2, G) for h in range(2 * HL) for G in (0, 1)
                    if (h % 2 == 0) == (G == 0)]
            work += [(h, 2) for h in range(HL)] + [(h, 3) for h in range(HL)]
            wo_after = {(h, 2): [(0, h)] for h in range(HL)}
            wo_after.update({(h, 3): [(1, h), (2, h)] for h in range(HL)})
            with tc.tile_pool(name="p4", bufs=4) as p4:
                pending = None
                for h, G in work:
                    strips = emit_scores(h, G)
                    if pending is not None:
                        emit_pv(*pending)
                    pending = (h, G, strips)
                    for woG, wong in wo_after.get((h, G), []):
                        emit_wo_chunk(woG, wong, p4)
                emit_pv(*pending)
                for ng in range(D // 512):
                    emit_wo_chunk(NG - 1, ng, p4, tail=True)

    nc.compile()
    return nc


def make_core_inputs(core, x, freqs, wq_w, wq_b, wkv_a_w, wkv_a_b, kv_norm_w,
                     wkv_b_w, wo_w, s_len):
    """Host-side shard + layout prep for one core."""
    b, g = core // TP, core % TP
    NB = s_len // 128
    heads = [TP * g + hh for hh in range(HL)]  # heads for TP rank g

    ins = {}
    # xt[s, p, 128k+c] = x[b, 128s+c, 128k+p]
    xb = np.ascontiguousarray(x[b, :s_len])                       # [S, D]
    xts = xb.reshape(NB, 128, KD, 128).transpose(0, 3, 2, 1)      # [s, p, k, c]
    ins["xt"] = to_bf16(np.ascontiguousarray(xts).reshape(NB, 128, D))
    # this rank's kv-latent shard: contiguous token blocks [4g .. 4g+NB/4)
    nloc = NB // 4
    ins["xkv"] = np.ascontiguousarray(ins["xt"][nloc * g:nloc * (g + 1)])

    # fused q+krope weight: rows = 4x nope(128), 4x qrope(64), krope(64)
    wq3 = wq_w.reshape(H, QK_HD, D)
    rows = [wq3[hg, :NOPE] for hg in heads] + [wq3[hg, NOPE:] for hg in heads]
    rows.append(wkv_a_w[KV:KV + ROPE])                            # krope [64, D]
    wq_sel = np.concatenate(rows, axis=0)                         # [832, D]
    wqt = wq_sel.T.reshape(KD, 128, QW).transpose(1, 0, 2)        # [p, k, 832]
    ins["wqn"] = to_bf16(np.ascontiguousarray(wqt[:, :, 0:512])
                         .reshape(128, KD * 512))
    ins["wqr"] = to_bf16(np.ascontiguousarray(wqt[:, :, 512:QW])
                         .reshape(128, KD * 320))

    wkvt = wkv_a_w[:KV].T.reshape(KD, 128, 512).transpose(1, 0, 2)
    ins["wkv"] = to_bf16(np.ascontiguousarray(wkvt).reshape(128, KD * 512))

    wkv_b3 = wkv_b_w.reshape(H, NOPE + V_HD, KV)
    # wbmt: per head, (wb_h * norm)^T in 4 chunks of [128c, 128d]
    wbt_cols = []
    for hg in heads:
        wb = wkv_b3[hg, :NOPE] * kv_norm_w[None, :]               # [128d, 512c]
        wbt_cols.append(wb.T.reshape(4, 128, 128).transpose(1, 0, 2).reshape(128, 512))
    ins["wbmt"] = to_bf16(np.concatenate(wbt_cols, axis=1))       # [128, 4*512]

    wvt_cols = []
    for hg in heads:
        wv = wkv_b3[hg, NOPE:] * kv_norm_w[None, :]               # [128d, 512c]
        wvt_cols.append(wv.T.reshape(4, 128, 128).transpose(1, 0, 2).reshape(128, 512))
    ins["wvt"] = to_bf16(np.concatenate(wvt_cols, axis=1))        # [128, 4*512]

    wo_cols = np.concatenate([wo_w[:, hg * V_HD:(hg + 1) * V_HD] for hg in heads],
                             axis=1)                              # [D, 512]
    wotl = wo_cols.T.reshape(HL, 128, D).transpose(1, 0, 2)       # [d, h, D]
    ins["wot"] = to_bf16(wotl.reshape(128, HL * D))

    # rope tables in [s-block(128), 64] free-pair layout
    fr = freqs[:s_len]                                            # [S, 32]
    cos2 = np.repeat(np.cos(fr), 2, axis=1).astype(np.float32)    # [S, 64]
    sin1 = np.sin(fr)
    sin2 = np.empty((s_len, ROPE), np.float32)
    sin2[:, 0::2] = -sin1
    sin2[:, 1::2] = sin1
    ins["cosq"] = to_bf16(
        cos2.reshape(NB, 128, 64).transpose(1, 0, 2).reshape(128, NB * 64))
    ins["sinq"] = to_bf16(
        sin2.reshape(NB, 128, 64).transpose(1, 0, 2).reshape(128, NB * 64))

    # transposed diagonal-block 0/1 mask for the S^T scores: [t, q], t > q
    # masked; applied as a post-exp multiply.
    ins["dmask"] = to_bf16(np.where(np.tril(np.ones((128, 128), bool), k=-1),
                                    np.float32(0.0), np.float32(1.0)))
    ins["identb"] = to_bf16(np.eye(128, dtype=np.float32))

    if np.any(wq_b != 0.0):
        rows_b = [wq_b.reshape(H, QK_HD)[hg, :NOPE] for hg in heads] + \
                 [wq_b.reshape(H, QK_HD)[hg, NOPE:] for hg in heads]
        rows_b.append(wkv_a_b[KV:KV + ROPE])
        ins["qb"] = to_bf16(np.concatenate(rows_b)[None, :])
    if np.any(wkv_a_b != 0.0):
        ins["kvb"] = to_bf16(wkv_a_b[:KV][None, :])
    return ins


_nc_cache = {}


def get_nc(s_len, q_bias, kv_bias):
    key = (s_len, q_bias, kv_bias)
    if key not in _nc_cache:
        _nc_cache[key] = build(s_len, q_bias, kv_bias)
    return _nc_cache[key]


def run_cores(inputs, s_len=S, trace=False):
    """Build per-core shards, run the SPMD kernel, return (out, results)."""
    x = np.asarray(inputs["x"], np.float32)
    freqs = np.asarray(inputs["freqs"], np.float32)
    wq_w = np.asarray(inputs["wq_w"], np.float32)
    wq_b = np.asarray(inputs["wq_b"], np.float32)
    wkv_a_w = np.asarray(inputs["wkv_a_w"], np.float32)
    wkv_a_b = np.asarray(inputs["wkv_a_b"], np.float32)
    kv_norm_w = np.asarray(inputs["kv_norm_w"], np.float32)
    wkv_b_w = np.asarray(inputs["wkv_b_w"], np.float32)
    wo_w = np.asarray(inputs["wo_w"], np.float32)
    wo_b = np.asarray(inputs["wo_b"], np.float32)

    q_bias = bool(np.any(wq_b != 0.0) or np.any(wkv_a_b[KV:] != 0.0))
    kv_bias = bool(np.any(wkv_a_b[:KV] != 0.0))
    nc = get_nc(s_len, q_bias, kv_bias)
    in_maps = [
        make_core_inputs(c, x, freqs, wq_w, wq_b, wkv_a_w, wkv_a_b, kv_norm_w,
                         wkv_b_w, wo_w, s_len)
        for c in range(N_CORES)
    ]
    res = bass_utils.run_bass_kernel_spmd(nc, in_maps, core_ids=list(range(N_CORES)),
                                          trace=trace)
    out = np.empty((B, s_len, D), np.float32)
    for b in range(B):
        p = [np.asarray(res.results[TP * b + g]["outT"], np.float32)
                .transpose(1, 0, 2).reshape(D, s_len).T
             for g in range(TP)]
        out[b] = (p[0] + p[1]) + (p[2] + p[3])
    out += wo_b[None, None, :]
    return out, res


def kernel(**inputs) -> np.ndarray:
    out, _ = run_cores(inputs, s_len=S, trace=False)
    return out



# revision 148
# speedup vs baseline: 1.0017x; 1.0017x over previous
"""MLA (DeepSeek-style multi-head latent attention) kernel for Trainium2.

Problem: nn_MultiHeadAttention_28243704939173
  B=2, S=2048, D=2048, H=16, KV_RANK=512, NOPE=128, ROPE=64, V_HD=128.

Sharding (8 NeuronCores): DP=2 over batch x TP=4 over heads (4 heads per
core). The kv-latent projection is sharded over the 4 TP ranks (each rank
projects S/4 contiguous token blocks, fed via the `xkv` input) and the
normalized kv^T shards are exchanged with an in-kernel AllGather over
replica groups [[0..3],[4..7]], overlapped with the q projection. Each
core emits its heads' partial wo projection transposed ([D, S]); the host
sums the 4 TP partials per batch element and adds wo_b.

Structure:
  * Per-head K/V materialization: k_eff_h^T = (wb_h norm) @ kv^T and
    v_eff_h = kv @ (wv_h norm)^T (both 128-d) are small GEMMs; scores then
    contract over 192 dims and PV over 128, instead of both in the 512-d
    latent space (~2.4x less PE work than the absorbed-latent form).
  * Everything runs in bf16 (full PE rate, half the DMA/SBUF, 2x DVE on
    packed copies); PSUM accumulation is fp32. Final rel err ~3e-3 vs the
    2e-2 gate.
  * q-nope is projected DIRECTLY TRANSPOSED (weight chunks stationary, x^T
    moving -> [d, tok] PSUM), so qnt needs no PE transposes; only the rope
    cols (q-rope + k-rope, 320 wide) go token-major for the DVE rope chain
    and get per-block PE transposes.
  * Phase 3 computes scores TRANSPOSED (keys on partitions): the Act exp
    output in SBUF is P^T and feeds the PV matmul directly -- no P
    transposes or PSUM->SBUF P copies. Off-diagonal strip pairs share one
    2-bank PSUM tile / one exp instruction; the causal triangle of the
    diagonal block is a post-exp 0/1 bf16 multiply. The softmax denominator
    is built OFF the PE: a DVE add-tree over the exp strips, a gpsimd
    partition_all_reduce (which also broadcasts), then reciprocal+multiply
    on the oT store. G0's tiny groups interleave with G1's, and wo chunks
    ride inside the attention loop as PE filler (sharing the po PSUM ring),
    so only the last 512-query group's wo runs after the attention drain.
  * The cost model serializes all DMA transfers on one engine pool, so the
    startup issues every front transfer on ONE queue in exact PE-consumption
    order (wkv chunk0, xkb0, remaining wkv, xkb1-3, rope-col weights, rope
    tables, x for q-group 0, nope weights); junk matmuls on a memset tile
    warm the PE pstate inside the initial DMA shadow, and pass-A blocks 0/1
    interleave at half-block granularity to ride the arrival curve. The kv^T
    AllGather (~80us hidden under pass B) feeds phase K, whose psums share
    the phase-3 rings inside one PSUM scope (no boundary stall).
"""
import numpy as np
from contextlib import ExitStack

import ml_dtypes

import concourse.bacc as bacc
import concourse.bass_isa as bass_isa
import concourse.mybir as mybir
import concourse.tile as tile
from concourse import bass_utils

F32 = mybir.dt.float32
BF16 = mybir.dt.bfloat16
AF = mybir.ActivationFunctionType
ALU = mybir.AluOpType

B, S, D = 2, 2048, 2048
H = 16
KV = 512
NOPE, ROPE = 128, 64
QK_HD = NOPE + ROPE
V_HD = 128
SCALE = float(QK_HD) ** -0.5
EPS = 1.1920929e-07
NEG = -1.0e5  # mask addend; NEG*SCALE ~ -7220 -> exp underflows to exactly 0
HL = 4        # local heads per core (TP degree 4)
TP = 4
N_CORES = 8
KD = D // 128   # contraction chunks over the model dim
QW = 832        # fused q-projection width: 4*128 nope + 4*64 qrope + 64 krope


def to_bf16(a: np.ndarray) -> np.ndarray:
    return np.ascontiguousarray(a).astype(ml_dtypes.bfloat16)


def build(s_len: int, q_bias: bool, kv_bias: bool):
    NB = s_len // 128

    nc = bacc.Bacc("TRN2", target_bir_lowering=False, debug=False)

    xt = nc.dram_tensor("xt", [NB, 128, D], BF16, kind="ExternalInput")
    xkv = nc.dram_tensor("xkv", [NB // 4, 128, D], BF16, kind="ExternalInput")
    wqn = nc.dram_tensor("wqn", [128, KD * 512], BF16, kind="ExternalInput")
    wqr = nc.dram_tensor("wqr", [128, KD * 320], BF16, kind="ExternalInput")
    wkv = nc.dram_tensor("wkv", [128, KD * 512], BF16, kind="ExternalInput")
    wbmt = nc.dram_tensor("wbmt", [128, HL * 512], BF16, kind="ExternalInput")
    wvt = nc.dram_tensor("wvt", [128, HL * 512], BF16, kind="ExternalInput")
    wot = nc.dram_tensor("wot", [128, HL * D], BF16, kind="ExternalInput")
    cosq = nc.dram_tensor("cosq", [128, NB * 64], BF16, kind="ExternalInput")
    sinq = nc.dram_tensor("sinq", [128, NB * 64], BF16, kind="ExternalInput")
    dmask = nc.dram_tensor("dmask", [128, 128], BF16, kind="ExternalInput")
    identb = nc.dram_tensor("identb", [128, 128], BF16, kind="ExternalInput")
    if q_bias:
        qb = nc.dram_tensor("qb", [1, QW], BF16, kind="ExternalInput")
    if kv_bias:
        kvb = nc.dram_tensor("kvb", [1, 512], BF16, kind="ExternalInput")
    outT = nc.dram_tensor("outT", [128, D // 128, s_len], BF16,
                          kind="ExternalOutput")

    with tile.TileContext(nc) as tc, ExitStack() as ctx:
        # ---------------- persistent tensors (whole kernel) -----------------
        pe = ctx.enter_context(tc.tile_pool(name="pe", bufs=1))
        identb_sb = pe.tile([128, 128], BF16, tag="identb_sb")
        dmask_sb = pe.tile([128, 128], BF16, tag="dmask_sb")
        qnt = pe.tile([128, HL * s_len], BF16, tag="qnt")       # [d, (h,q)]
        qpet = [pe.tile([128, s_len], BF16, tag=f"qpet{pp}", name=f"qpet{pp}")
                for pp in range(2)]                             # [(2h,r), q]
        kpet = pe.tile([128, s_len], BF16, tag="kpet")          # [r x2, t]
        kvt = pe.tile([128, 4 * s_len], BF16, tag="kvt")        # [c, (cc,t)]
        wbmt_sb = pe.tile([128, HL * 512], BF16, tag="wbmt_sb")
        wvt_sb = pe.tile([128, HL * 512], BF16, tag="wvt_sb")

        # ========== Phase 12: fused kv-latent + q/k-rope projections ========
        with tc.tile_pool(name="p12w", bufs=1) as p12w, \
                tc.tile_pool(name="p12", bufs=3) as p12, \
                tc.tile_pool(name="p12s", bufs=6) as p12s, \
                tc.tile_pool(name="pst", bufs=2, space="PSUM") as pst, \
                tc.tile_pool(name="psq2", bufs=2, space="PSUM") as psq2:
            # Startup DMAs: the cost model serializes all transfers on one
            # DMA-engine resource, so the GLOBAL arrival order must track the
            # PE's consumption order: wkv chunk0 + xkb0 first (pass A block
            # 0), the remaining wkv chunks while block 0 runs, then xkb1-3,
            # then wq groups + xg0 blocks for pass B, misc last. Orders are
            # interleaved across the SP/Act queues (shared HWDGE round-robin).
            NLOC = NB // 4
            xkb = [p12.tile([128, 2048], BF16, tag="xkb", name=f"xkb{ss}",
                            bufs=4) for ss in range(NLOC)]
            wkv_splits = [(0, 1), (1, 1), (2, 2), (4, 2), (6, 2),
                          (8, 2), (10, 2), (12, 2), (14, 2)]
            wkvg = {}
            wkvc = [None] * KD
            for k0, nk_ in wkv_splits:
                wt = p12w.tile([128, 512 * nk_], BF16, tag=f"wkvg{k0}",
                               name=f"wkvg{k0}")
                wkvg[k0] = (wt, nk_)
                for kk in range(nk_):
                    wkvc[k0 + kk] = wt[:, 512 * kk:512 * (kk + 1)]

            def dma_wkv(eng, k0):
                wt, nk_ = wkvg[k0]
                eng.dma_start(wt[:], wkv.ap()[:, 512 * k0:512 * (k0 + nk_)])

            cosq_sb = p12w.tile([128, NB * 64], BF16, tag="cosq_sb")
            sinq_sb = p12w.tile([128, NB * 64], BF16, tag="sinq_sb")
            # q weights split by column type: rope cols are needed first
            # (group-0 ropes run right after pass A), nope cols ~8us later
            wqng = [p12w.tile([128, 2 * 512], BF16, tag="wqng",
                              name=f"wqng{i}", bufs=8) for i in range(8)]
            wqrg = [p12w.tile([128, 2 * 320], BF16, tag="wqrg",
                              name=f"wqrg{i}", bufs=8) for i in range(8)]
            wqnc = [wqng[k // 2][:, 512 * (k % 2):512 * (k % 2 + 1)]
                    for k in range(KD)]
            wqrc = [wqrg[k // 2][:, 320 * (k % 2):320 * (k % 2 + 1)]
                    for k in range(KD)]

            xg0 = p12.tile([128, 4 * 2048], BF16, tag="xg", bufs=3, name="xg0")
            xg0v = xg0[:].rearrange("p (b k c) -> p b k c", b=4, k=KD)

            # PE warm-up: junk matmuls on a memset tile during the initial
            # DMA wait, so the pstate ramp (HAM on real hw) finishes in the
            # shadow and pass A runs at full rate from its first matmul.
            jt = p12w.tile([128, 320], BF16, tag="jt")
            nc.vector.memset(jt[:], 0.0)

            def emit_junk(n):
                for _ in range(n):
                    pj = psq2.tile([128, 320], F32, tag="pq2")
                    nc.tensor.matmul(pj[:], jt[:, 0:128], jt[:],
                                     start=True, stop=True,
                                     skip_group_check=True)

            emit_junk(14)

            # -- the ordered startup sequence: ONE queue (SP), strict demand
            # order, so nothing jumps ahead in the serialized transfer order.
            # (identb rides the Pool SWDGE: 128x128, negligible.)
            dma_wkv(nc.sync, 0)
            nc.sync.dma_start(xkb[0][:], xkv.ap()[0][:, :])
            dma_wkv(nc.sync, 1)
            dma_wkv(nc.sync, 2)
            dma_wkv(nc.sync, 4)
            nc.sync.dma_start(xkb[1][:], xkv.ap()[1][:, :])
            dma_wkv(nc.sync, 6)
            dma_wkv(nc.sync, 8)
            dma_wkv(nc.sync, 10)
            dma_wkv(nc.sync, 12)
            dma_wkv(nc.sync, 14)
            nc.sync.dma_start(xkb[2][:], xkv.ap()[2][:, :])
            nc.sync.dma_start(wqrg[0][:], wqr.ap()[:, 0:640])
            nc.sync.dma_start(xkb[3][:], xkv.ap()[3][:, :])
            for i in range(1, 8):
                nc.sync.dma_start(wqrg[i][:],
                                  wqr.ap()[:, 640 * i:640 * (i + 1)])
            # rope tables: group-0's slice first, the rest off the front
            nc.sync.dma_start(cosq_sb[:, 0:256], cosq.ap()[:, 0:256])
            nc.sync.dma_start(sinq_sb[:, 0:256], sinq.ap()[:, 0:256])
            nc.sync.dma_start(xg0v[:, 0, :, :], xt.ap()[0][:, :])
            for b in range(1, 4):
                nc.sync.dma_start(xg0v[:, b, :, :], xt.ap()[b][:, :])
            for i in range(8):
                nc.sync.dma_start(wqng[i][:],
                                  wqn.ap()[:, 1024 * i:1024 * (i + 1)])
            nc.sync.dma_start(cosq_sb[:, 256:NB * 64], cosq.ap()[:, 256:NB * 64])
            nc.sync.dma_start(sinq_sb[:, 256:NB * 64], sinq.ap()[:, 256:NB * 64])
            nc.sync.dma_start(dmask_sb[:], dmask.ap()[:])
            nc.gpsimd.dma_start(identb_sb[:], identb.ap()[:])
            nc.sync.dma_start(wbmt_sb[:], wbmt.ap()[:])
            nc.sync.dma_start(wvt_sb[:], wvt.ap()[:])
            if q_bias or kv_bias:
                ones1 = p12w.tile([1, 128], BF16, tag="ones1")
                nc.vector.memset(ones1[:], 1.0)
            if q_bias:
                qb_sb = p12w.tile([1, QW], BF16, tag="qb_sb")
                nc.scalar.dma_start(qb_sb[:], qb.ap()[:])
                ones512 = p12w.tile([1, 512], BF16, tag="ones512")
                nc.vector.memset(ones512[:], 1.0)
            if kv_bias:
                kvb_sb = p12w.tile([1, 512], BF16, tag="kvb_sb")
                nc.scalar.dma_start(kvb_sb[:], kvb.ap()[:])

            qnt4 = qnt[:].rearrange("p (h n c) -> p h n c", h=HL, n=NB)
            kvt4 = kvt[:].rearrange("p (cc n c) -> p cc n c", cc=4, n=NB)

            # ---- pass A: kv latent for this rank's NB/4 blocks (sharded) ----
            kvsh = p12w.tile([128, 4 * 128 * NLOC], BF16, tag="kvsh")
            kvsh4 = kvsh[:].rearrange("p (cc n c) -> p cc n c", cc=4, n=NLOC)

            def emit_tr_kv(st):
                ss, kv_bf = st
                ptc = pst.tile([128, 512], BF16, tag="ptc")
                for cc in range(4):
                    nc.tensor.transpose(ptc[:, 128 * cc:128 * (cc + 1)],
                                        kv_bf[:, 128 * cc:128 * (cc + 1)],
                                        identb_sb[:])
                nc.vector.tensor_copy(
                    kvsh4[:, :, ss, :],
                    ptc[:].rearrange("p (cc c) -> p cc c", cc=4))

            prevkv = None

            pkvs = {}

            def emit_passA_mm(ss, ka, kb):
                if ka == 0:
                    pkvs[ss] = pskv.tile([128, 512], F32, tag="pkv",
                                         name=f"pkv{ss}")
                pkv = pkvs[ss]
                for k in range(ka, kb):
                    # filler so the warm PE never idles (or resets its
                    # pstate) while the weight-chunk stream catches up
                    if ss == 0 and k in (2, 4, 6, 10, 13):
                        emit_junk(2)
                    lhs = xkb[ss][:, 128 * k:128 * (k + 1)]
                    nc.tensor.matmul(pkv[:], lhs, wkvc[k][:],
                                     start=(k == 0),
                                     stop=(k == KD - 1 and not kv_bias),
                                     skip_group_check=True)
                if kb == KD and kv_bias:
                    nc.tensor.matmul(pkv[:], ones1[:], kvb_sb[:],
                                     start=False, stop=True,
                                     skip_group_check=True)

            def emit_passA(ss):
                nonlocal prevkv
                pkv = pkvs[ss]
                if prevkv is not None:
                    emit_tr_kv(prevkv)
                kvtile = p12.tile([128, 512], F32, tag="kvtile")
                nc.scalar.copy(kvtile[:], pkv[:])
                sq = p12.tile([128, 512], F32, tag="sq", bufs=3)
                msq = p12s.tile([128, 1], F32, tag="msq")
                nc.scalar.activation(sq[:], kvtile[:], AF.Square, bias=0.0,
                                     scale=1.0, accum_out=msq[:])
                ms2 = p12s.tile([128, 1], F32, tag="ms2")
                nc.vector.tensor_scalar(ms2[:], msq[:], 1.0 / KV, EPS, ALU.mult,
                                        ALU.add)
                srt = p12s.tile([128, 1], F32, tag="srt")
                nc.scalar.sqrt(srt[:], ms2[:])
                rrt = p12s.tile([128, 1], F32, tag="rrt")
                nc.vector.reciprocal(rrt[:], srt[:])
                kv_bf = p12.tile([128, 512], BF16, tag="kv_bf", bufs=4)
                nc.vector.tensor_scalar(kv_bf[:], kvtile[:], rrt[:], None,
                                        ALU.mult)
                # krope: bf16 rope chain on this rank's block (sharded)
                prevkv = (ss, kv_bf)

            def emit_collective(ccd, in_b):
                out_b = ccd.tile([TP, 128, 4 * 128 * NLOC], BF16,
                                 name="kv_ag_out")
                nc.gpsimd.dma_start(in_b[:], kvsh[:])
                nc.gpsimd.collective_compute(
                    "AllGather", ALU.bypass,
                    replica_groups=[[0, 1, 2, 3], [4, 5, 6, 7]],
                    ins=[in_b.opt()], outs=[out_b.opt()])
                for r in range(TP):
                    nc.gpsimd.dma_start(
                        kvt4[:, :, NLOC * r:NLOC * (r + 1), :],
                        out_b[r].rearrange("p (cc n c) -> p cc n c",
                                           cc=4, n=NLOC))

            prev = None

            def emit_tr(st):
                """PE transposes for the rope cols of block st (lagged one)."""
                s, qro_bf = st
                ptp = pst.tile([128, 512], BF16, tag="ptc")
                for pp in range(2):
                    nc.tensor.transpose(ptp[:, 128 * pp:128 * (pp + 1)],
                                        qro_bf[:, 128 * pp:128 * (pp + 1)],
                                        identb_sb[:])
                    nc.vector.tensor_copy(qpet[pp][:, 128 * s:128 * (s + 1)],
                                          ptp[:, 128 * pp:128 * (pp + 1)])
                # kpeT duplicated on both partition halves (the rope matmul
                # pairs it with either half of qpet); DVE cannot cross
                # partitions, so transpose twice with an explicit placement.
                nc.tensor.transpose(ptp[0:64, 256:384], qro_bf[:, 256:320],
                                    identb_sb[:], tile_position=(0, 0))
                nc.tensor.transpose(ptp[64:128, 256:384], qro_bf[:, 256:320],
                                    identb_sb[:], tile_position=(0, 64))
                nc.vector.tensor_copy(kpet[:, 128 * s:128 * (s + 1)],
                                      ptp[:, 256:384])

            def emit_rope(s, xg4):
                """Rope q-cols (+krope) for one 128-token block, token-major."""
                nonlocal prev
                b = s % 4
                pq2 = psq2.tile([128, 320], F32, tag="pq2")
                for k in range(KD):
                    nc.tensor.matmul(pq2[:], xg4[:, b, k, :], wqrc[k][:],
                                     start=(k == 0),
                                     stop=(k == KD - 1 and not q_bias),
                                     skip_group_check=True)
                if q_bias:
                    nc.tensor.matmul(pq2[:], ones1[:], qb_sb[:, 512:832],
                                     start=False, stop=True,
                                     skip_group_check=True)
                if prev is not None:
                    emit_tr(prev)
                # whole rope chain in bf16: 2x DVE modes, no final copy (qr
                # stays alive for the lagged transposes, hence bufs=3)
                qr = p12.tile([128, 320], BF16, tag="qr", bufs=4)
                nc.scalar.copy(qr[:], pq2[:])
                qsw = p12.tile([128, 320], BF16, tag="qsw")
                a3 = qr[:].rearrange("p (i two) -> p i two", two=2)
                w3 = qsw[:].rearrange("p (i two) -> p i two", two=2)
                nc.vector.tensor_copy(w3[:, :, 0:1], a3[:, :, 1:2])
                nc.vector.tensor_copy(w3[:, :, 1:2], a3[:, :, 0:1])
                cs = cosq_sb[:, 64 * s:64 * (s + 1)]
                sn = sinq_sb[:, 64 * s:64 * (s + 1)]
                for r in range(5):
                    rsl = qr[:, 64 * r:64 * (r + 1)]
                    ssl = qsw[:, 64 * r:64 * (r + 1)]
                    nc.vector.tensor_mul(rsl, rsl, cs)
                    nc.vector.tensor_mul(ssl, ssl, sn)
                    nc.vector.tensor_add(rsl, rsl, ssl)
                prev = (s, qr)

            def alloc_xg(g):
                """Prefetch the next group's x blocks (ring of 3)."""
                xg = p12.tile([128, 4 * 2048], BF16, tag="xg", bufs=3,
                              name=f"xg{g}")
                xg4 = xg[:].rearrange("p (b k c) -> p b k c", b=4, k=KD)
                for b in range(4):
                    nc.sync.dma_start(xg4[:, b, :, :],
                                      xt.ap()[4 * g + b][:, :])
                return xg4

            def emit_nope(g, xg4, psq1, with_rope):
                """q-nope for one 512-token group: directly transposed
                (weights stationary, x^T moving -> [d, tok] PSUM, no PE
                transposes). Group 0 goes block-by-block with a k-outer loop
                so its x/weight demand is paced with the startup DMA stream;
                later groups (data resident) use group-wide N=512 matmuls."""
                pqts = [psq1.tile([128, 512], F32, tag="pqt",
                                  name=f"pqt{g}_{h}") for h in range(HL)]
                if g == 0:
                    for b in range(4):
                        for k in range(KD):
                            for h in range(HL):
                                nc.tensor.matmul(
                                    pqts[h][:, 128 * b:128 * (b + 1)],
                                    wqnc[k][:, 128 * h:128 * (h + 1)],
                                    xg4[:, b, k, :],
                                    start=(k == 0),
                                    stop=(k == KD - 1 and not q_bias),
                                    skip_group_check=True)
                        if q_bias:
                            for h in range(HL):
                                nc.tensor.matmul(
                                    pqts[h][:, 128 * b:128 * (b + 1)],
                                    qb_sb[:, 128 * h:128 * (h + 1)], ones1[:],
                                    start=False, stop=True,
                                    skip_group_check=True)
                else:
                    for h in range(HL):
                        for k in range(KD):
                            nc.tensor.matmul(
                                pqts[h][:],
                                wqnc[k][:, 128 * h:128 * (h + 1)],
                                xg4[:, :, k, :],
                                start=(k == 0),
                                stop=(k == KD - 1 and not q_bias),
                                skip_group_check=True)
                        if q_bias:
                            nc.tensor.matmul(
                                pqts[h][:],
                                qb_sb[:, 128 * h:128 * (h + 1)], ones512[:],
                                start=False, stop=True, skip_group_check=True)
                        if with_rope:
                            emit_rope(4 * g + h, xg4)
                        # copy as soon as this head's psum completes, so the
                        # next group's pqt ring slot frees ~10us earlier
                        nc.scalar.copy(qnt4[:, h, 4 * g:4 * (g + 1), :]
                                       .rearrange("p n c -> p (n c)"),
                                       pqts[h][:])
                if g == 0:
                    if with_rope:
                        for b in range(4):
                            emit_rope(4 * g + b, xg4)
                    for h in range(HL):
                        nc.scalar.copy(qnt4[:, h, 4 * g:4 * (g + 1), :]
                                       .rearrange("p n c -> p (n c)"),
                                       pqts[h][:])

            # order: pass A; collective; group-0 ropes (fed by the early
            # rope-col weights); group-0 nope; groups 1-3 (nope+rope)
            with tc.tile_pool(name="ccd", bufs=1, space="DRAM") as ccd:
                in_b = ccd.tile([128, 4 * 128 * NLOC], BF16,
                                name="kv_ag_in")
                with tc.tile_pool(name="pskv", bufs=2, space="PSUM") as pskv:
                    # blocks 0/1 interleaved at half-block granularity:
                    # block 1 (fed by the early xkb1 transfer) fills block
                    # 0's weight-arrival stalls
                    emit_passA_mm(0, 0, 8)
                    emit_passA_mm(1, 0, 8)
                    emit_passA_mm(0, 8, KD)
                    emit_passA(0)
                    emit_passA_mm(1, 8, KD)
                    emit_passA(1)
                    for i in range(2, NLOC):
                        emit_passA_mm(i, 0, KD)
                        emit_passA(i)
                    emit_tr_kv(prevkv)
                emit_collective(ccd, in_b)
                for b in range(4):
                    emit_rope(b, xg0v)
                with tc.tile_pool(name="psq1", bufs=4, space="PSUM") as psq1:
                    nxt = alloc_xg(1)
                    emit_nope(0, xg0v, psq1, with_rope=False)
                    for g in range(1, NB // 4):
                        cur = nxt
                        if g < NB // 4 - 1:
                            nxt = alloc_xg(g + 1)
                        emit_nope(g, cur, psq1, with_rope=True)
                    emit_tr(prev)

        # ---------------- persistent tensors (post-P12) ---------------------
        pl = ctx.enter_context(tc.tile_pool(name="pl", bufs=1))
        keff = pl.tile([128, HL * s_len], BF16, tag="keff")   # [d, (h,t)]
        veff = pl.tile([128, HL * s_len], BF16, tag="veff")   # [t, (h,j*d)]
        ot_bf = pl.tile([128, HL * s_len], BF16, tag="ot_bf")  # [d, (h,q)]
        wot_sb = pl.tile([128, HL * D], BF16, tag="wot_sb")
        nc.scalar.dma_start(wot_sb[:], wot.ap()[:])

        # ========== Phase K: materialize per-head K/V =======================
        NG = s_len // 512

        # ========== Phases K+3 (one PSUM scope, no boundary stall) =========
        # Scores are computed transposed (keys on partitions, queries on the
        # free dim), so the Act exp output in SBUF IS P^T and feeds the PV
        # matmul directly -- no transposes, no PSUM->SBUF P copies. The
        # softmax denominator is built OFF the PE: a DVE add-tree over the
        # exp strips followed by a gpsimd partition_all_reduce (which also
        # broadcasts), then a single DVE divide on the oT store. Off-diagonal
        # strips are paired into 2-bank PSUM tiles so one exp instruction
        # covers both. The causal mask inside the diagonal 128x128 block is a
        # post-exp multiply by a 0/1 bf16 triangle (exp(x-1e5*m) == exp(x)*t).
        with tc.tile_pool(name="expp", bufs=1) as expp, \
                tc.tile_pool(name="accp", bufs=4) as accp, \
                tc.tile_pool(name="rzp", bufs=4) as rzp, \
                tc.tile_pool(name="pssp", bufs=2, space="PSUM") as pssp, \
                tc.tile_pool(name="pssd", bufs=2, space="PSUM") as pssd, \
                tc.tile_pool(name="ps3o", bufs=2, space="PSUM") as ps3o:
            # ---- phase K: per-head K/V materialization (psums share the
            # pssd ring so the K->3 transition reuses warm banks) ----
            for h in range(HL):
                for tg in range(NG):
                    pk = pssd.tile([128, 512], F32, tag="psd")
                    for cc in range(4):
                        nc.tensor.matmul(
                            pk[:],
                            wbmt_sb[:, 512 * h + 128 * cc:512 * h + 128 * (cc + 1)],
                            kvt[:, s_len * cc + 512 * tg:s_len * cc + 512 * (tg + 1)],
                            start=(cc == 0), stop=(cc == 3))
                    nc.scalar.copy(
                        keff[:, s_len * h + 512 * tg:s_len * h + 512 * (tg + 1)],
                        pk[:])
            # veff: one [t, (h,d)] psum per 128-token block -- N=512 matmuls
            # across all 4 heads at once (4x fewer instructions than per-head)
            wvt4 = wvt_sb[:].rearrange("p (h c) -> p h c", h=HL)
            veff3 = veff[:].rearrange("p (h t) -> p h t", h=HL)
            for j in range(NB):
                pv = ps3o.tile([128, 512], F32, tag="po")
                for cc in range(4):
                    nc.tensor.matmul(
                        pv[:],
                        kvt[:, s_len * cc + 128 * j:s_len * cc + 128 * (j + 1)],
                        wvt4[:, :, 128 * cc:128 * (cc + 1)],
                        start=(cc == 0), stop=(cc == 3),
                        skip_group_check=True)
                nc.vector.tensor_copy(
                    veff3[:, :, 128 * j:128 * (j + 1)],
                    pv[:].rearrange("p (h c) -> p h c", h=HL))

            def emit_scores(h, G):
                """Transposed scores + exp; off-diag strips paired per PSUM."""
                strips = []
                hb = 64 * (h % 2)
                q0 = s_len * h + 512 * G
                k0 = s_len * h

                def score_mms(dst, j, c0):
                    nc.tensor.matmul(
                        dst,
                        keff[:, k0 + 128 * j:k0 + 128 * (j + 1)],
                        qnt[:, q0 + 128 * c0:q0 + 512],
                        start=True, stop=False, skip_group_check=True)
                    nc.tensor.matmul(
                        dst,
                        kpet[hb:hb + 64, 128 * j:128 * (j + 1)],
                        qpet[h // 2][hb:hb + 64,
                                     512 * G + 128 * c0:512 * (G + 1)],
                        start=False, stop=True, skip_group_check=True)

                for jp in range(2 * G):
                    ps = pssp.tile([128, 1024], F32, tag="psp")
                    for u in range(2):
                        score_mms(ps[:, 512 * u:512 * (u + 1)], 2 * jp + u, 0)
                    et = expp.tile([128, 1024], BF16, tag="etp", bufs=15)
                    nc.scalar.activation(et[:], ps[:], AF.Exp, bias=0.0,
                                         scale=SCALE)
                    strips.append(("pair", jp, et))
                for ii in range(4):
                    j = 4 * G + ii
                    ps = pssd.tile([128, 512], F32, tag="psd")
                    score_mms(ps[:, 128 * ii:512], j, ii)
                    et = expp.tile([128, 512], BF16, tag="etd", bufs=11)
                    nc.scalar.activation(et[:, 128 * ii:512], ps[:, 128 * ii:512],
                                         AF.Exp, bias=0.0, scale=SCALE)
                    # zero the masked (t > q) triangle of the diagonal block
                    nc.vector.tensor_mul(et[:, 128 * ii:128 * (ii + 1)],
                                         et[:, 128 * ii:128 * (ii + 1)],
                                         dmask_sb[:])
                    strips.append(("diag", ii, et))
                return strips

            def emit_pv(h, G, strips):
                """PV accumulation + off-PE softmax denominator, /Z on store."""
                po = ps3o.tile([128, 512], F32, tag="po")
                acc = accp.tile([128, 512], BF16, tag="acc")
                last = 4 * G + 3
                k0 = s_len * h
                nacc = 0
                for kind, idx, et in strips:
                    if kind == "pair":
                        for u in range(2):
                            j = 2 * idx + u
                            nc.tensor.matmul(
                                po[:],
                                veff[:, k0 + 128 * j:k0 + 128 * (j + 1)],
                                et[:, 512 * u:512 * (u + 1)],
                                start=(j == 0), stop=(j == last),
                                skip_group_check=True)
                        if idx == 0:
                            nc.vector.tensor_add(acc[:], et[:, 0:512],
                                                 et[:, 512:1024])
                        else:
                            nc.vector.tensor_add(acc[:], acc[:], et[:, 0:512])
                            nc.vector.tensor_add(acc[:], acc[:], et[:, 512:1024])
                        nacc += 2
                    else:
                        ii = idx
                        j = 4 * G + ii
                        nc.tensor.matmul(
                            po[:, 128 * ii:512],
                            veff[:, k0 + 128 * j:k0 + 128 * (j + 1)],
                            et[:, 128 * ii:512],
                            start=(j == 0), stop=(j == last),
                            skip_group_check=True)
                        if nacc == 0:
                            nc.vector.tensor_copy(acc[:], et[:, 0:512])
                        else:
                            nc.vector.tensor_add(acc[:, 128 * ii:512],
                                                 acc[:, 128 * ii:512],
                                                 et[:, 128 * ii:512])
                        nacc += 1
                zr = rzp.tile([128, 512], F32, tag="zr")
                nc.gpsimd.partition_all_reduce(zr[:], acc[:], 128,
                                               bass_isa.ReduceOp.add)
                rz = rzp.tile([128, 512], F32, tag="rz")
                nc.vector.reciprocal(rz[:], zr[:])
                nc.vector.tensor_mul(
                    ot_bf[:, s_len * h + 512 * G:s_len * h + 512 * (G + 1)],
                    po[:], rz[:])

            # ---- phase 4 chunks (wo projection), interleaved into the
            # attention loop as PE filler: wo(G-1, ng) rides between the
            # score groups of G, sharing the "po" PSUM ring ----
            def emit_wo_chunk(G, ng, p4, tail=False):
                last_chunk = (G == NG - 1 and ng == D // 512 - 1)
                osb = p4.tile([128, 2048], BF16, tag="osb")
                for nn in range(4):
                    n = 4 * ng + nn
                    # after attention drains, the diag-score ring is idle --
                    # alternating psums halves the ring-handoff waits
                    pool_ = pssd if tail and nn % 2 else ps3o
                    pw = pool_.tile([128, 512], F32,
                                    tag="psd" if tail and nn % 2 else "po")
                    for h in range(HL):
                        nc.tensor.matmul(
                            pw[:],
                            wot_sb[:, D * h + 128 * n:D * h + 128 * (n + 1)],
                            ot_bf[:, s_len * h + 512 * G:s_len * h + 512 * (G + 1)],
                            start=(h == 0), stop=(h == HL - 1))
                    nc.scalar.copy(osb[:, 512 * nn:512 * (nn + 1)], pw[:])
                    if last_chunk:
                        eng = nc.sync if nn % 2 == 0 else nc.scalar
                        eng.dma_start(
                            outT.ap()[:, 4 * ng + nn, 512 * G:512 * (G + 1)],
                            osb[:, 512 * nn:512 * (nn + 1)])
                if not last_chunk:
                    nc.sync.dma_start(
                        outT.ap()[:, 4 * ng:4 * (ng + 1),
                                  512 * G:512 * (G + 1)],
                        osb[:].rearrange("p (n q) -> p n q", n=4))

            # G0's tiny groups interleave with G1's so the exp latency of the
            # short strips hides behind bigger PE work; wo chunks ride along
            # once their group's last divide is in flight.
            work = [(h // 2, G) for h in range(2 * HL) for G in (0, 1)
                    if (h % 2 == 0) == (G == 0)]
            work += [(h, 2) for h in range(HL)] + [(h, 3) for h in range(HL)]
            wo_after = {(h, 2): [(0, h)] for h in range(HL)}
            wo_after.update({(h, 3): [(1, h), (2, h)] for h in range(HL)})
            with tc.tile_pool(name="p4", bufs=4) as p4:
                pending = None
                for h, G in work:
                    strips = emit_scores(h, G)
                    if pending is not None:
                        emit_pv(*pending)
                    pending = (h, G, strips)
                    for woG, wong in wo_after.get((h, G), []):
                        emit_wo_chunk(woG, wong, p4)
                emit_pv(*pending)
                for ng in range(D // 512):
                    emit_wo_chunk(NG - 1, ng, p4, tail=True)

    nc.compile()
    return nc


def make_core_inputs(core, x, freqs, wq_w, wq_b, wkv_a_w, wkv_a_b, kv_norm_w,
                     wkv_b_w, wo_w, s_len):
    """Host-side shard + layout prep for one core."""
    b, g = core // TP, core % TP
    NB = s_len // 128
    heads = [TP * g + hh for hh in range(HL)]  # heads for TP rank g

    ins = {}
    # xt[s, p, 128k+c] = x[b, 128s+c, 128k+p]
    xb = np.ascontiguousarray(x[b, :s_len])                       # [S, D]
    xts = xb.reshape(NB, 128, KD, 128).transpose(0, 3, 2, 1)      # [s, p, k, c]
    ins["xt"] = to_bf16(np.ascontiguousarray(xts).reshape(NB, 128, D))
    # this rank's kv-latent shard: contiguous token blocks [4g .. 4g+NB/4)
    nloc = NB // 4
    ins["xkv"] = np.ascontiguousarray(ins["xt"][nloc * g:nloc * (g + 1)])

    # fused q+krope weight: rows = 4x nope(128), 4x qrope(64), krope(64)
    wq3 = wq_w.reshape(H, QK_HD, D)
    rows = [wq3[hg, :NOPE] for hg in heads] + [wq3[hg, NOPE:] for hg in heads]
    rows.append(wkv_a_w[KV:KV + ROPE])                            # krope [64, D]
    wq_sel = np.concatenate(rows, axis=0)                         # [832, D]
    wqt = wq_sel.T.reshape(KD, 128, QW).transpose(1, 0, 2)        # [p, k, 832]
    ins["wqn"] = to_bf16(np.ascontiguousarray(wqt[:, :, 0:512])
                         .reshape(128, KD * 512))
    ins["wqr"] = to_bf16(np.ascontiguousarray(wqt[:, :, 512:QW])
                         .reshape(128, KD * 320))

    wkvt = wkv_a_w[:KV].T.reshape(KD, 128, 512).transpose(1, 0, 2)
    ins["wkv"] = to_bf16(np.ascontiguousarray(wkvt).reshape(128, KD * 512))

    wkv_b3 = wkv_b_w.reshape(H, NOPE + V_HD, KV)
    # wbmt: per head, (wb_h * norm)^T in 4 chunks of [128c, 128d]
    wbt_cols = []
    for hg in heads:
        wb = wkv_b3[hg, :NOPE] * kv_norm_w[None, :]               # [128d, 512c]
        wbt_cols.append(wb.T.reshape(4, 128, 128).transpose(1, 0, 2).reshape(128, 512))
    ins["wbmt"] = to_bf16(np.concatenate(wbt_cols, axis=1))       # [128, 4*512]

    wvt_cols = []
    for hg in heads:
        wv = wkv_b3[hg, NOPE:] * kv_norm_w[None, :]               # [128d, 512c]
        wvt_cols.append(wv.T.reshape(4, 128, 128).transpose(1, 0, 2).reshape(128, 512))
    ins["wvt"] = to_bf16(np.concatenate(wvt_cols, axis=1))        # [128, 4*512]

    wo_cols = np.concatenate([wo_w[:, hg * V_HD:(hg + 1) * V_HD] for hg in heads],
                             axis=1)                              # [D, 512]
    wotl = wo_cols.T.reshape(HL, 128, D).transpose(1, 0, 2)       # [d, h, D]
    ins["wot"] = to_bf16(wotl.reshape(128, HL * D))

    # rope tables in [s-block(128), 64] free-pair layout
    fr = freqs[:s_len]                                            # [S, 32]
    cos2 = np.repeat(np.cos(fr), 2, axis=1).astype(np.float32)    # [S, 64]
    sin1 = np.sin(fr)
    sin2 = np.empty((s_len, ROPE), np.float32)
    sin2[:, 0::2] = -sin1
    sin2[:, 1::2] = sin1
    ins["cosq"] = to_bf16(
        cos2.reshape(NB, 128, 64).transpose(1, 0, 2).reshape(128, NB * 64))
    ins["sinq"] = to_bf16(
        sin2.reshape(NB, 128, 64).transpose(1, 0, 2).reshape(128, NB * 64))

    # transposed diagonal-block 0/1 mask for the S^T scores: [t, q], t > q
    # masked; applied as a post-exp multiply.
    ins["dmask"] = to_bf16(np.where(np.tril(np.ones((128, 128), bool), k=-1),
                                    np.float32(0.0), np.float32(1.0)))
    ins["identb"] = to_bf16(np.eye(128, dtype=np.float32))

    if np.any(wq_b != 0.0):
        rows_b = [wq_b.reshape(H, QK_HD)[hg, :NOPE] for hg in heads] + \
                 [wq_b.reshape(H, QK_HD)[hg, NOPE:] for hg in heads]
        rows_b.append(wkv_a_b[KV:KV + ROPE])
        ins["qb"] = to_bf16(np.concatenate(rows_b)[None, :])
    if np.any(wkv_a_b != 0.0):
        ins["kvb"] = to_bf16(wkv_a_b[:KV][None, :])
    return ins


_nc_cache = {}


def get_nc(s_len, q_bias, kv_bias):
    key = (s_len, q_bias, kv_bias)
    if key not in _nc_cache:
        _nc_cache[key] = build(s_len, q_bias, kv_bias)
    return _nc_cache[key]


def run_cores(inputs, s_len=S, trace=False):
    """Build per-core shards, run the SPMD kernel, return (out, results)."""
    x = np.asarray(inputs["x"], np.float32)
    freqs = np.asarray(inputs["freqs"], np.float32)
    wq_w = np.asarray(inputs["wq_w"], np.float32)
    wq_b = np.asarray(inputs["wq_b"], np.float32)
    wkv_a_w = np.asarray(inputs["wkv_a_w"], np.float32)
    wkv_a_b = np.asarray(inputs["wkv_a_b"], np.float32)
    kv_norm_w = np.asarray(inputs["kv_norm_w"], np.float32)
    wkv_b_w = np.asarray(inputs["wkv_b_w"], np.float32)
    wo_w = np.asarray(inputs["wo_w"], np.float32)
    wo_b = np.asarray(inputs["wo_b"], np.float32)

    q_bias = bool(np.any(wq_b != 0.0) or np.any(wkv_a_b[KV:] != 0.0))
    kv_bias = bool(np.any(wkv_a_b[:KV] != 0.0))
    nc = get_nc(s_len, q_bias, kv_bias)
    in_maps = [
        make_core_inputs(c, x, freqs, wq_w, wq_b, wkv_a_w, wkv_a_b, kv_norm_w,
                         wkv_b_w, wo_w, s_len)
        for c in range(N_CORES)
    ]
    res = bass_utils.run_bass_kernel_spmd(nc, in_maps, core_ids=list(range(N_CORES)),
                                          trace=trace)
    out = np.empty((B, s_len, D), np.float32)
    for b in range(B):
        p = [np.asarray(res.results[TP * b + g]["outT"], np.float32)
                .transpose(1, 0, 2).reshape(D, s_len).T
             for g in range(TP)]
        out[b] = (p[0] + p[1]) + (p[2] + p[3])
    out += wo_b[None, None, :]
    return out, res


def kernel(**inputs) -> np.ndarray:
    out, _ = run_cores(inputs, s_len=S, trace=False)
    return out



# revision 150
# speedup vs baseline: 1.0026x; 1.0009x over previous
"""MLA (DeepSeek-style multi-head latent attention) kernel for Trainium2.

Problem: nn_MultiHeadAttention_28243704939173
  B=2, S=2048, D=2048, H=16, KV_RANK=512, NOPE=128, ROPE=64, V_HD=128.

Sharding (8 NeuronCores): DP=2 over batch x TP=4 over heads (4 heads per
core). The kv-latent projection is sharded over the 4 TP ranks (each rank
projects S/4 contiguous token blocks, fed via the `xkv` input) and the
normalized kv^T shards are exchanged with an in-kernel AllGather over
replica groups [[0..3],[4..7]], overlapped with the q projection. Each
core emits its heads' partial wo projection transposed ([D, S]); the host
sums the 4 TP partials per batch element and adds wo_b.

Structure:
  * Per-head K/V materialization: k_eff_h^T = (wb_h norm) @ kv^T and
    v_eff_h = kv @ (wv_h norm)^T (both 128-d) are small GEMMs; scores then
    contract over 192 dims and PV over 128, instead of both in the 512-d
    latent space (~2.4x less PE work than the absorbed-latent form).
  * Everything runs in bf16 (full PE rate, half the DMA/SBUF, 2x DVE on
    packed copies); PSUM accumulation is fp32. Final rel err ~3e-3 vs the
    2e-2 gate.
  * q-nope is projected DIRECTLY TRANSPOSED (weight chunks stationary, x^T
    moving -> [d, tok] PSUM), so qnt needs no PE transposes; only the rope
    cols (q-rope + k-rope, 320 wide) go token-major for the DVE rope chain
    and get per-block PE transposes.
  * Phase 3 computes scores TRANSPOSED (keys on partitions): the Act exp
    output in SBUF is P^T and feeds the PV matmul directly -- no P
    transposes or PSUM->SBUF P copies. Off-diagonal strip pairs share one
    2-bank PSUM tile / one exp instruction; the causal triangle of the
    diagonal block is a post-exp 0/1 bf16 multiply. The softmax denominator
    is built OFF the PE: a DVE add-tree over the exp strips, a gpsimd
    partition_all_reduce (which also broadcasts), then reciprocal+multiply
    on the oT store. G0's tiny groups interleave with G1's, and wo chunks
    ride inside the attention loop as PE filler (sharing the po PSUM ring),
    so only the last 512-query group's wo runs after the attention drain.
  * The cost model serializes all DMA transfers on one engine pool, so the
    startup issues every front transfer on ONE queue in exact PE-consumption
    order (wkv chunk0, xkb0, remaining wkv, xkb1-3, rope-col weights, rope
    tables, x for q-group 0, nope weights); junk matmuls on a memset tile
    warm the PE pstate inside the initial DMA shadow, and pass-A blocks 0/1
    interleave at half-block granularity to ride the arrival curve. The kv^T
    AllGather (~80us hidden under pass B) feeds phase K, whose psums share
    the phase-3 rings inside one PSUM scope (no boundary stall).
"""
import numpy as np
from contextlib import ExitStack

import ml_dtypes

import concourse.bacc as bacc
import concourse.bass_isa as bass_isa
import concourse.mybir as mybir
import concourse.tile as tile
from concourse import bass_utils

F32 = mybir.dt.float32
BF16 = mybir.dt.bfloat16
AF = mybir.ActivationFunctionType
ALU = mybir.AluOpType

B, S, D = 2, 2048, 2048
H = 16
KV = 512
NOPE, ROPE = 128, 64
QK_HD = NOPE + ROPE
V_HD = 128
SCALE = float(QK_HD) ** -0.5
EPS = 1.1920929e-07
NEG = -1.0e5  # mask addend; NEG*SCALE ~ -7220 -> exp underflows to exactly 0
HL = 4        # local heads per core (TP degree 4)
TP = 4
N_CORES = 8
KD = D // 128   # contraction chunks over the model dim
QW = 832        # fused q-projection width: 4*128 nope + 4*64 qrope + 64 krope


def to_bf16(a: np.ndarray) -> np.ndarray:
    return np.ascontiguousarray(a).astype(ml_dtypes.bfloat16)


def build(s_len: int, q_bias: bool, kv_bias: bool):
    NB = s_len // 128

    nc = bacc.Bacc("TRN2", target_bir_lowering=False, debug=False)

    xt = nc.dram_tensor("xt", [NB, 128, D], BF16, kind="ExternalInput")
    xkv = nc.dram_tensor("xkv", [NB // 4, 128, D], BF16, kind="ExternalInput")
    wqn = nc.dram_tensor("wqn", [128, KD * 512], BF16, kind="ExternalInput")
    wqr = nc.dram_tensor("wqr", [128, KD * 320], BF16, kind="ExternalInput")
    wkv = nc.dram_tensor("wkv", [128, KD * 512], BF16, kind="ExternalInput")
    wbmt = nc.dram_tensor("wbmt", [128, HL * 512], BF16, kind="ExternalInput")
    wvt = nc.dram_tensor("wvt", [128, HL * 512], BF16, kind="ExternalInput")
    wot = nc.dram_tensor("wot", [128, HL * D], BF16, kind="ExternalInput")
    cosq = nc.dram_tensor("cosq", [128, NB * 64], BF16, kind="ExternalInput")
    sinq = nc.dram_tensor("sinq", [128, NB * 64], BF16, kind="ExternalInput")
    dmask = nc.dram_tensor("dmask", [128, 128], BF16, kind="ExternalInput")
    identb = nc.dram_tensor("identb", [128, 128], BF16, kind="ExternalInput")
    if q_bias:
        qb = nc.dram_tensor("qb", [1, QW], BF16, kind="ExternalInput")
    if kv_bias:
        kvb = nc.dram_tensor("kvb", [1, 512], BF16, kind="ExternalInput")
    outT = nc.dram_tensor("outT", [128, D // 128, s_len], BF16,
                          kind="ExternalOutput")

    with tile.TileContext(nc) as tc, ExitStack() as ctx:
        # ---------------- persistent tensors (whole kernel) -----------------
        pe = ctx.enter_context(tc.tile_pool(name="pe", bufs=1))
        identb_sb = pe.tile([128, 128], BF16, tag="identb_sb")
        dmask_sb = pe.tile([128, 128], BF16, tag="dmask_sb")
        qnt = pe.tile([128, HL * s_len], BF16, tag="qnt")       # [d, (h,q)]
        qpet = [pe.tile([128, s_len], BF16, tag=f"qpet{pp}", name=f"qpet{pp}")
                for pp in range(2)]                             # [(2h,r), q]
        kpet = pe.tile([128, s_len], BF16, tag="kpet")          # [r x2, t]
        kvt = pe.tile([128, 4 * s_len], BF16, tag="kvt")        # [c, (cc,t)]
        wbmt_sb = pe.tile([128, HL * 512], BF16, tag="wbmt_sb")
        wvt_sb = pe.tile([128, HL * 512], BF16, tag="wvt_sb")

        # ========== Phase 12: fused kv-latent + q/k-rope projections ========
        with tc.tile_pool(name="p12w", bufs=1) as p12w, \
                tc.tile_pool(name="p12", bufs=3) as p12, \
                tc.tile_pool(name="p12s", bufs=6) as p12s, \
                tc.tile_pool(name="pst", bufs=2, space="PSUM") as pst, \
                tc.tile_pool(name="psq2", bufs=2, space="PSUM") as psq2:
            # Startup DMAs: the cost model serializes all transfers on one
            # DMA-engine resource, so the GLOBAL arrival order must track the
            # PE's consumption order: wkv chunk0 + xkb0 first (pass A block
            # 0), the remaining wkv chunks while block 0 runs, then xkb1-3,
            # then wq groups + xg0 blocks for pass B, misc last. Orders are
            # interleaved across the SP/Act queues (shared HWDGE round-robin).
            NLOC = NB // 4
            xkb = [p12.tile([128, 2048], BF16, tag="xkb", name=f"xkb{ss}",
                            bufs=4) for ss in range(NLOC)]
            wkv_splits = [(0, 1), (1, 1), (2, 2), (4, 2), (6, 2),
                          (8, 2), (10, 2), (12, 2), (14, 2)]
            wkvg = {}
            wkvc = [None] * KD
            for k0, nk_ in wkv_splits:
                wt = p12w.tile([128, 512 * nk_], BF16, tag=f"wkvg{k0}",
                               name=f"wkvg{k0}")
                wkvg[k0] = (wt, nk_)
                for kk in range(nk_):
                    wkvc[k0 + kk] = wt[:, 512 * kk:512 * (kk + 1)]

            def dma_wkv(eng, k0):
                wt, nk_ = wkvg[k0]
                eng.dma_start(wt[:], wkv.ap()[:, 512 * k0:512 * (k0 + nk_)])

            cosq_sb = p12w.tile([128, NB * 64], BF16, tag="cosq_sb")
            sinq_sb = p12w.tile([128, NB * 64], BF16, tag="sinq_sb")
            # q weights split by column type: rope cols are needed first
            # (group-0 ropes run right after pass A), nope cols ~8us later
            wqng = [p12w.tile([128, 2 * 512], BF16, tag="wqng",
                              name=f"wqng{i}", bufs=8) for i in range(8)]
            wqrg = [p12w.tile([128, 2 * 320], BF16, tag="wqrg",
                              name=f"wqrg{i}", bufs=8) for i in range(8)]
            wqnc = [wqng[k // 2][:, 512 * (k % 2):512 * (k % 2 + 1)]
                    for k in range(KD)]
            wqrc = [wqrg[k // 2][:, 320 * (k % 2):320 * (k % 2 + 1)]
                    for k in range(KD)]

            xg0 = p12.tile([128, 4 * 2048], BF16, tag="xg", bufs=3, name="xg0")
            xg0v = xg0[:].rearrange("p (b k c) -> p b k c", b=4, k=KD)

            # PE warm-up: junk matmuls on a memset tile during the initial
            # DMA wait, so the pstate ramp (HAM on real hw) finishes in the
            # shadow and pass A runs at full rate from its first matmul.
            jt = p12w.tile([128, 320], BF16, tag="jt")
            nc.vector.memset(jt[:], 0.0)

            def emit_junk(n):
                for _ in range(n):
                    pj = psq2.tile([128, 320], F32, tag="pq2")
                    nc.tensor.matmul(pj[:], jt[:, 0:128], jt[:],
                                     start=True, stop=True,
                                     skip_group_check=True)

            emit_junk(14)

            # -- the ordered startup sequence: ONE queue (SP), strict demand
            # order, so nothing jumps ahead in the serialized transfer order.
            # (identb rides the Pool SWDGE: 128x128, negligible.)
            dma_wkv(nc.sync, 0)
            nc.sync.dma_start(xkb[0][:, 0:1024], xkv.ap()[0][:, 0:1024])
            nc.sync.dma_start(xkb[0][:, 1024:2048], xkv.ap()[0][:, 1024:2048])
            dma_wkv(nc.sync, 1)
            dma_wkv(nc.sync, 2)
            dma_wkv(nc.sync, 4)
            nc.sync.dma_start(xkb[1][:, 0:1024], xkv.ap()[1][:, 0:1024])
            nc.sync.dma_start(xkb[1][:, 1024:2048], xkv.ap()[1][:, 1024:2048])
            dma_wkv(nc.sync, 6)
            dma_wkv(nc.sync, 8)
            dma_wkv(nc.sync, 10)
            dma_wkv(nc.sync, 12)
            dma_wkv(nc.sync, 14)
            nc.sync.dma_start(xkb[2][:, 0:1024], xkv.ap()[2][:, 0:1024])
            nc.sync.dma_start(xkb[2][:, 1024:2048], xkv.ap()[2][:, 1024:2048])
            nc.sync.dma_start(wqrg[0][:], wqr.ap()[:, 0:640])
            nc.sync.dma_start(xkb[3][:, 0:1024], xkv.ap()[3][:, 0:1024])
            nc.sync.dma_start(xkb[3][:, 1024:2048], xkv.ap()[3][:, 1024:2048])
            for i in range(1, 8):
                nc.sync.dma_start(wqrg[i][:],
                                  wqr.ap()[:, 640 * i:640 * (i + 1)])
            # rope tables: group-0's slice first, the rest off the front
            nc.sync.dma_start(cosq_sb[:, 0:256], cosq.ap()[:, 0:256])
            nc.sync.dma_start(sinq_sb[:, 0:256], sinq.ap()[:, 0:256])
            nc.sync.dma_start(xg0v[:, 0, 0:8, :], xt.ap()[0][:, 0:1024])
            nc.sync.dma_start(xg0v[:, 0, 8:16, :], xt.ap()[0][:, 1024:2048])
            for b in range(1, 4):
                nc.sync.dma_start(xg0v[:, b, :, :], xt.ap()[b][:, :])
            for i in range(8):
                nc.sync.dma_start(wqng[i][:],
                                  wqn.ap()[:, 1024 * i:1024 * (i + 1)])
            nc.sync.dma_start(cosq_sb[:, 256:NB * 64], cosq.ap()[:, 256:NB * 64])
            nc.sync.dma_start(sinq_sb[:, 256:NB * 64], sinq.ap()[:, 256:NB * 64])
            nc.sync.dma_start(dmask_sb[:], dmask.ap()[:])
            nc.gpsimd.dma_start(identb_sb[:], identb.ap()[:])
            nc.sync.dma_start(wbmt_sb[:], wbmt.ap()[:])
            nc.sync.dma_start(wvt_sb[:], wvt.ap()[:])
            if q_bias or kv_bias:
                ones1 = p12w.tile([1, 128], BF16, tag="ones1")
                nc.vector.memset(ones1[:], 1.0)
            if q_bias:
                qb_sb = p12w.tile([1, QW], BF16, tag="qb_sb")
                nc.scalar.dma_start(qb_sb[:], qb.ap()[:])
                ones512 = p12w.tile([1, 512], BF16, tag="ones512")
                nc.vector.memset(ones512[:], 1.0)
            if kv_bias:
                kvb_sb = p12w.tile([1, 512], BF16, tag="kvb_sb")
                nc.scalar.dma_start(kvb_sb[:], kvb.ap()[:])

            qnt4 = qnt[:].rearrange("p (h n c) -> p h n c", h=HL, n=NB)
            kvt4 = kvt[:].rearrange("p (cc n c) -> p cc n c", cc=4, n=NB)

            # ---- pass A: kv latent for this rank's NB/4 blocks (sharded) ----
            kvsh = p12w.tile([128, 4 * 128 * NLOC], BF16, tag="kvsh")
            kvsh4 = kvsh[:].rearrange("p (cc n c) -> p cc n c", cc=4, n=NLOC)

            def emit_tr_kv(st):
                ss, kv_bf = st
                ptc = pst.tile([128, 512], BF16, tag="ptc")
                for cc in range(4):
                    nc.tensor.transpose(ptc[:, 128 * cc:128 * (cc + 1)],
                                        kv_bf[:, 128 * cc:128 * (cc + 1)],
                                        identb_sb[:])
                nc.vector.tensor_copy(
                    kvsh4[:, :, ss, :],
                    ptc[:].rearrange("p (cc c) -> p cc c", cc=4))

            prevkv = None

            pkvs = {}

            def emit_passA_mm(ss, ka, kb):
                if ka == 0:
                    pkvs[ss] = pskv.tile([128, 512], F32, tag="pkv",
                                         name=f"pkv{ss}")
                pkv = pkvs[ss]
                for k in range(ka, kb):
                    # filler so the warm PE never idles (or resets its
                    # pstate) while the weight-chunk stream catches up
                    if ss == 0 and k in (2, 4, 6, 10, 13):
                        emit_junk(2)
                    lhs = xkb[ss][:, 128 * k:128 * (k + 1)]
                    nc.tensor.matmul(pkv[:], lhs, wkvc[k][:],
                                     start=(k == 0),
                                     stop=(k == KD - 1 and not kv_bias),
                                     skip_group_check=True)
                if kb == KD and kv_bias:
                    nc.tensor.matmul(pkv[:], ones1[:], kvb_sb[:],
                                     start=False, stop=True,
                                     skip_group_check=True)

            def emit_passA(ss):
                nonlocal prevkv
                pkv = pkvs[ss]
                if prevkv is not None:
                    emit_tr_kv(prevkv)
                kvtile = p12.tile([128, 512], F32, tag="kvtile")
                nc.scalar.copy(kvtile[:], pkv[:])
                sq = p12.tile([128, 512], F32, tag="sq", bufs=3)
                msq = p12s.tile([128, 1], F32, tag="msq")
                nc.scalar.activation(sq[:], kvtile[:], AF.Square, bias=0.0,
                                     scale=1.0, accum_out=msq[:])
                ms2 = p12s.tile([128, 1], F32, tag="ms2")
                nc.vector.tensor_scalar(ms2[:], msq[:], 1.0 / KV, EPS, ALU.mult,
                                        ALU.add)
                srt = p12s.tile([128, 1], F32, tag="srt")
                nc.scalar.sqrt(srt[:], ms2[:])
                rrt = p12s.tile([128, 1], F32, tag="rrt")
                nc.vector.reciprocal(rrt[:], srt[:])
                kv_bf = p12.tile([128, 512], BF16, tag="kv_bf", bufs=4)
                nc.vector.tensor_scalar(kv_bf[:], kvtile[:], rrt[:], None,
                                        ALU.mult)
                # krope: bf16 rope chain on this rank's block (sharded)
                prevkv = (ss, kv_bf)

            def emit_collective(ccd, in_b):
                out_b = ccd.tile([TP, 128, 4 * 128 * NLOC], BF16,
                                 name="kv_ag_out")
                nc.gpsimd.dma_start(in_b[:], kvsh[:])
                nc.gpsimd.collective_compute(
                    "AllGather", ALU.bypass,
                    replica_groups=[[0, 1, 2, 3], [4, 5, 6, 7]],
                    ins=[in_b.opt()], outs=[out_b.opt()])
                for r in range(TP):
                    nc.gpsimd.dma_start(
                        kvt4[:, :, NLOC * r:NLOC * (r + 1), :],
                        out_b[r].rearrange("p (cc n c) -> p cc n c",
                                           cc=4, n=NLOC))

            prev = None

            def emit_tr(st):
                """PE transposes for the rope cols of block st (lagged one)."""
                s, qro_bf = st
                ptp = pst.tile([128, 512], BF16, tag="ptc")
                for pp in range(2):
                    nc.tensor.transpose(ptp[:, 128 * pp:128 * (pp + 1)],
                                        qro_bf[:, 128 * pp:128 * (pp + 1)],
                                        identb_sb[:])
                    nc.vector.tensor_copy(qpet[pp][:, 128 * s:128 * (s + 1)],
                                          ptp[:, 128 * pp:128 * (pp + 1)])
                # kpeT duplicated on both partition halves (the rope matmul
                # pairs it with either half of qpet); DVE cannot cross
                # partitions, so transpose twice with an explicit placement.
                nc.tensor.transpose(ptp[0:64, 256:384], qro_bf[:, 256:320],
                                    identb_sb[:], tile_position=(0, 0))
                nc.tensor.transpose(ptp[64:128, 256:384], qro_bf[:, 256:320],
                                    identb_sb[:], tile_position=(0, 64))
                nc.vector.tensor_copy(kpet[:, 128 * s:128 * (s + 1)],
                                      ptp[:, 256:384])

            def emit_rope(s, xg4):
                """Rope q-cols (+krope) for one 128-token block, token-major."""
                nonlocal prev
                b = s % 4
                pq2 = psq2.tile([128, 320], F32, tag="pq2")
                for k in range(KD):
                    nc.tensor.matmul(pq2[:], xg4[:, b, k, :], wqrc[k][:],
                                     start=(k == 0),
                                     stop=(k == KD - 1 and not q_bias),
                                     skip_group_check=True)
                if q_bias:
                    nc.tensor.matmul(pq2[:], ones1[:], qb_sb[:, 512:832],
                                     start=False, stop=True,
                                     skip_group_check=True)
                if prev is not None:
                    emit_tr(prev)
                # whole rope chain in bf16: 2x DVE modes, no final copy (qr
                # stays alive for the lagged transposes, hence bufs=3)
                qr = p12.tile([128, 320], BF16, tag="qr", bufs=4)
                nc.scalar.copy(qr[:], pq2[:])
                qsw = p12.tile([128, 320], BF16, tag="qsw")
                a3 = qr[:].rearrange("p (i two) -> p i two", two=2)
                w3 = qsw[:].rearrange("p (i two) -> p i two", two=2)
                nc.vector.tensor_copy(w3[:, :, 0:1], a3[:, :, 1:2])
                nc.vector.tensor_copy(w3[:, :, 1:2], a3[:, :, 0:1])
                cs = cosq_sb[:, 64 * s:64 * (s + 1)]
                sn = sinq_sb[:, 64 * s:64 * (s + 1)]
                for r in range(5):
                    rsl = qr[:, 64 * r:64 * (r + 1)]
                    ssl = qsw[:, 64 * r:64 * (r + 1)]
                    nc.vector.tensor_mul(rsl, rsl, cs)
                    nc.vector.tensor_mul(ssl, ssl, sn)
                    nc.vector.tensor_add(rsl, rsl, ssl)
                prev = (s, qr)

            def alloc_xg(g):
                """Prefetch the next group's x blocks (ring of 3)."""
                xg = p12.tile([128, 4 * 2048], BF16, tag="xg", bufs=3,
                              name=f"xg{g}")
                xg4 = xg[:].rearrange("p (b k c) -> p b k c", b=4, k=KD)
                for b in range(4):
                    nc.sync.dma_start(xg4[:, b, :, :],
                                      xt.ap()[4 * g + b][:, :])
                return xg4

            def emit_nope(g, xg4, psq1, with_rope):
                """q-nope for one 512-token group: directly transposed
                (weights stationary, x^T moving -> [d, tok] PSUM, no PE
                transposes). Group 0 goes block-by-block with a k-outer loop
                so its x/weight demand is paced with the startup DMA stream;
                later groups (data resident) use group-wide N=512 matmuls."""
                pqts = [psq1.tile([128, 512], F32, tag="pqt",
                                  name=f"pqt{g}_{h}") for h in range(HL)]
                if g == 0:
                    for b in range(4):
                        for k in range(KD):
                            for h in range(HL):
                                nc.tensor.matmul(
                                    pqts[h][:, 128 * b:128 * (b + 1)],
                                    wqnc[k][:, 128 * h:128 * (h + 1)],
                                    xg4[:, b, k, :],
                                    start=(k == 0),
                                    stop=(k == KD - 1 and not q_bias),
                                    skip_group_check=True)
                        if q_bias:
                            for h in range(HL):
                                nc.tensor.matmul(
                                    pqts[h][:, 128 * b:128 * (b + 1)],
                                    qb_sb[:, 128 * h:128 * (h + 1)], ones1[:],
                                    start=False, stop=True,
                                    skip_group_check=True)
                else:
                    for h in range(HL):
                        for k in range(KD):
                            nc.tensor.matmul(
                                pqts[h][:],
                                wqnc[k][:, 128 * h:128 * (h + 1)],
                                xg4[:, :, k, :],
                                start=(k == 0),
                                stop=(k == KD - 1 and not q_bias),
                                skip_group_check=True)
                        if q_bias:
                            nc.tensor.matmul(
                                pqts[h][:],
                                qb_sb[:, 128 * h:128 * (h + 1)], ones512[:],
                                start=False, stop=True, skip_group_check=True)
                        if with_rope:
                            emit_rope(4 * g + h, xg4)
                        # copy as soon as this head's psum completes, so the
                        # next group's pqt ring slot frees ~10us earlier
                        nc.scalar.copy(qnt4[:, h, 4 * g:4 * (g + 1), :]
                                       .rearrange("p n c -> p (n c)"),
                                       pqts[h][:])
                if g == 0:
                    if with_rope:
                        for b in range(4):
                            emit_rope(4 * g + b, xg4)
                    for h in range(HL):
                        nc.scalar.copy(qnt4[:, h, 4 * g:4 * (g + 1), :]
                                       .rearrange("p n c -> p (n c)"),
                                       pqts[h][:])

            # order: pass A; collective; group-0 ropes (fed by the early
            # rope-col weights); group-0 nope; groups 1-3 (nope+rope)
            with tc.tile_pool(name="ccd", bufs=1, space="DRAM") as ccd:
                in_b = ccd.tile([128, 4 * 128 * NLOC], BF16,
                                name="kv_ag_in")
                with tc.tile_pool(name="pskv", bufs=2, space="PSUM") as pskv:
                    # blocks 0/1 interleaved at half-block granularity:
                    # block 1 (fed by the early xkb1 transfer) fills block
                    # 0's weight-arrival stalls
                    emit_passA_mm(0, 0, 8)
                    emit_passA_mm(1, 0, 8)
                    emit_passA_mm(0, 8, KD)
                    emit_passA(0)
                    emit_passA_mm(1, 8, KD)
                    emit_passA(1)
                    for i in range(2, NLOC):
                        emit_passA_mm(i, 0, KD)
                        emit_passA(i)
                    emit_tr_kv(prevkv)
                emit_collective(ccd, in_b)
                for b in range(4):
                    emit_rope(b, xg0v)
                with tc.tile_pool(name="psq1", bufs=4, space="PSUM") as psq1:
                    nxt = alloc_xg(1)
                    emit_nope(0, xg0v, psq1, with_rope=False)
                    for g in range(1, NB // 4):
                        cur = nxt
                        if g < NB // 4 - 1:
                            nxt = alloc_xg(g + 1)
                        emit_nope(g, cur, psq1, with_rope=True)
                    emit_tr(prev)

        # ---------------- persistent tensors (post-P12) ---------------------
        pl = ctx.enter_context(tc.tile_pool(name="pl", bufs=1))
        keff = pl.tile([128, HL * s_len], BF16, tag="keff")   # [d, (h,t)]
        veff = pl.tile([128, HL * s_len], BF16, tag="veff")   # [t, (h,j*d)]
        ot_bf = pl.tile([128, HL * s_len], BF16, tag="ot_bf")  # [d, (h,q)]
        wot_sb = pl.tile([128, HL * D], BF16, tag="wot_sb")
        nc.scalar.dma_start(wot_sb[:], wot.ap()[:])

        # ========== Phase K: materialize per-head K/V =======================
        NG = s_len // 512

        # ========== Phases K+3 (one PSUM scope, no boundary stall) =========
        # Scores are computed transposed (keys on partitions, queries on the
        # free dim), so the Act exp output in SBUF IS P^T and feeds the PV
        # matmul directly -- no transposes, no PSUM->SBUF P copies. The
        # softmax denominator is built OFF the PE: a DVE add-tree over the
        # exp strips followed by a gpsimd partition_all_reduce (which also
        # broadcasts), then a single DVE divide on the oT store. Off-diagonal
        # strips are paired into 2-bank PSUM tiles so one exp instruction
        # covers both. The causal mask inside the diagonal 128x128 block is a
        # post-exp multiply by a 0/1 bf16 triangle (exp(x-1e5*m) == exp(x)*t).
        with tc.tile_pool(name="expp", bufs=1) as expp, \
                tc.tile_pool(name="accp", bufs=4) as accp, \
                tc.tile_pool(name="rzp", bufs=4) as rzp, \
                tc.tile_pool(name="pssp", bufs=2, space="PSUM") as pssp, \
                tc.tile_pool(name="pssd", bufs=2, space="PSUM") as pssd, \
                tc.tile_pool(name="ps3o", bufs=2, space="PSUM") as ps3o:
            # ---- phase K: per-head K/V materialization (psums share the
            # pssd ring so the K->3 transition reuses warm banks) ----
            for h in range(HL):
                for tg in range(NG):
                    pk = pssd.tile([128, 512], F32, tag="psd")
                    for cc in range(4):
                        nc.tensor.matmul(
                            pk[:],
                            wbmt_sb[:, 512 * h + 128 * cc:512 * h + 128 * (cc + 1)],
                            kvt[:, s_len * cc + 512 * tg:s_len * cc + 512 * (tg + 1)],
                            start=(cc == 0), stop=(cc == 3))
                    nc.scalar.copy(
                        keff[:, s_len * h + 512 * tg:s_len * h + 512 * (tg + 1)],
                        pk[:])
            # veff: one [t, (h,d)] psum per 128-token block -- N=512 matmuls
            # across all 4 heads at once (4x fewer instructions than per-head)
            wvt4 = wvt_sb[:].rearrange("p (h c) -> p h c", h=HL)
            veff3 = veff[:].rearrange("p (h t) -> p h t", h=HL)
            for j in range(NB):
                pv = ps3o.tile([128, 512], F32, tag="po")
                for cc in range(4):
                    nc.tensor.matmul(
                        pv[:],
                        kvt[:, s_len * cc + 128 * j:s_len * cc + 128 * (j + 1)],
                        wvt4[:, :, 128 * cc:128 * (cc + 1)],
                        start=(cc == 0), stop=(cc == 3),
                        skip_group_check=True)
                nc.vector.tensor_copy(
                    veff3[:, :, 128 * j:128 * (j + 1)],
                    pv[:].rearrange("p (h c) -> p h c", h=HL))

            def emit_scores(h, G):
                """Transposed scores + exp; off-diag strips paired per PSUM."""
                strips = []
                hb = 64 * (h % 2)
                q0 = s_len * h + 512 * G
                k0 = s_len * h

                def score_mms(dst, j, c0):
                    nc.tensor.matmul(
                        dst,
                        keff[:, k0 + 128 * j:k0 + 128 * (j + 1)],
                        qnt[:, q0 + 128 * c0:q0 + 512],
                        start=True, stop=False, skip_group_check=True)
                    nc.tensor.matmul(
                        dst,
                        kpet[hb:hb + 64, 128 * j:128 * (j + 1)],
                        qpet[h // 2][hb:hb + 64,
                                     512 * G + 128 * c0:512 * (G + 1)],
                        start=False, stop=True, skip_group_check=True)

                for jp in range(2 * G):
                    ps = pssp.tile([128, 1024], F32, tag="psp")
                    for u in range(2):
                        score_mms(ps[:, 512 * u:512 * (u + 1)], 2 * jp + u, 0)
                    et = expp.tile([128, 1024], BF16, tag="etp", bufs=15)
                    nc.scalar.activation(et[:], ps[:], AF.Exp, bias=0.0,
                                         scale=SCALE)
                    strips.append(("pair", jp, et))
                for ii in range(4):
                    j = 4 * G + ii
                    ps = pssd.tile([128, 512], F32, tag="psd")
                    score_mms(ps[:, 128 * ii:512], j, ii)
                    et = expp.tile([128, 512], BF16, tag="etd", bufs=11)
                    nc.scalar.activation(et[:, 128 * ii:512], ps[:, 128 * ii:512],
                                         AF.Exp, bias=0.0, scale=SCALE)
                    # zero the masked (t > q) triangle of the diagonal block
                    nc.vector.tensor_mul(et[:, 128 * ii:128 * (ii + 1)],
                                         et[:, 128 * ii:128 * (ii + 1)],
                                         dmask_sb[:])
                    strips.append(("diag", ii, et))
                return strips

            def emit_pv(h, G, strips):
                """PV accumulation + off-PE softmax denominator, /Z on store."""
                po = ps3o.tile([128, 512], F32, tag="po")
                acc = accp.tile([128, 512], BF16, tag="acc")
                last = 4 * G + 3
                k0 = s_len * h
                nacc = 0
                for kind, idx, et in strips:
                    if kind == "pair":
                        for u in range(2):
                            j = 2 * idx + u
                            nc.tensor.matmul(
                                po[:],
                                veff[:, k0 + 128 * j:k0 + 128 * (j + 1)],
                                et[:, 512 * u:512 * (u + 1)],
                                start=(j == 0), stop=(j == last),
                                skip_group_check=True)
                        if idx == 0:
                            nc.vector.tensor_add(acc[:], et[:, 0:512],
                                                 et[:, 512:1024])
                        else:
                            nc.vector.tensor_add(acc[:], acc[:], et[:, 0:512])
                            nc.vector.tensor_add(acc[:], acc[:], et[:, 512:1024])
                        nacc += 2
                    else:
                        ii = idx
                        j = 4 * G + ii
                        nc.tensor.matmul(
                            po[:, 128 * ii:512],
                            veff[:, k0 + 128 * j:k0 + 128 * (j + 1)],
                            et[:, 128 * ii:512],
                            start=(j == 0), stop=(j == last),
                            skip_group_check=True)
                        if nacc == 0:
                            nc.vector.tensor_copy(acc[:], et[:, 0:512])
                        else:
                            nc.vector.tensor_add(acc[:, 128 * ii:512],
                                                 acc[:, 128 * ii:512],
                                                 et[:, 128 * ii:512])
                        nacc += 1
                zr = rzp.tile([128, 512], F32, tag="zr")
                nc.gpsimd.partition_all_reduce(zr[:], acc[:], 128,
                                               bass_isa.ReduceOp.add)
                rz = rzp.tile([128, 512], F32, tag="rz")
                nc.vector.reciprocal(rz[:], zr[:])
                nc.vector.tensor_mul(
                    ot_bf[:, s_len * h + 512 * G:s_len * h + 512 * (G + 1)],
                    po[:], rz[:])

            # ---- phase 4 chunks (wo projection), interleaved into the
            # attention loop as PE filler: wo(G-1, ng) rides between the
            # score groups of G, sharing the "po" PSUM ring ----
            def emit_wo_chunk(G, ng, p4, tail=False):
                last_chunk = (G == NG - 1 and ng == D // 512 - 1)
                osb = p4.tile([128, 2048], BF16, tag="osb")
                for nn in range(4):
                    n = 4 * ng + nn
                    # after attention drains, the diag-score ring is idle --
                    # alternating psums halves the ring-handoff waits
                    pool_ = pssd if tail and nn % 2 else ps3o
                    pw = pool_.tile([128, 512], F32,
                                    tag="psd" if tail and nn % 2 else "po")
                    for h in range(HL):
                        nc.tensor.matmul(
                            pw[:],
                            wot_sb[:, D * h + 128 * n:D * h + 128 * (n + 1)],
                            ot_bf[:, s_len * h + 512 * G:s_len * h + 512 * (G + 1)],
                            start=(h == 0), stop=(h == HL - 1))
                    nc.scalar.copy(osb[:, 512 * nn:512 * (nn + 1)], pw[:])
                    if last_chunk:
                        eng = nc.sync if nn % 2 == 0 else nc.scalar
                        eng.dma_start(
                            outT.ap()[:, 4 * ng + nn, 512 * G:512 * (G + 1)],
                            osb[:, 512 * nn:512 * (nn + 1)])
                if not last_chunk:
                    nc.sync.dma_start(
                        outT.ap()[:, 4 * ng:4 * (ng + 1),
                                  512 * G:512 * (G + 1)],
                        osb[:].rearrange("p (n q) -> p n q", n=4))

            # G0's tiny groups interleave with G1's so the exp latency of the
            # short strips hides behind bigger PE work; wo chunks ride along
            # once their group's last divide is in flight.
            work = [(h // 2, G) for h in range(2 * HL) for G in (0, 1)
                    if (h % 2 == 0) == (G == 0)]
            work += [(h, 2) for h in range(HL)] + [(h, 3) for h in range(HL)]
            wo_after = {(h, 2): [(0, h)] for h in range(HL)}
            wo_after.update({(h, 3): [(1, h), (2, h)] for h in range(HL)})
            with tc.tile_pool(name="p4", bufs=4) as p4:
                pending = None
                for h, G in work:
                    strips = emit_scores(h, G)
                    if pending is not None:
                        emit_pv(*pending)
                    pending = (h, G, strips)
                    for woG, wong in wo_after.get((h, G), []):
                        emit_wo_chunk(woG, wong, p4)
                emit_pv(*pending)
                for ng in range(D // 512):
                    emit_wo_chunk(NG - 1, ng, p4, tail=True)

    nc.compile()
    return nc


def make_core_inputs(core, x, freqs, wq_w, wq_b, wkv_a_w, wkv_a_b, kv_norm_w,
                     wkv_b_w, wo_w, s_len):
    """Host-side shard + layout prep for one core."""
    b, g = core // TP, core % TP
    NB = s_len // 128
    heads = [TP * g + hh for hh in range(HL)]  # heads for TP rank g

    ins = {}
    # xt[s, p, 128k+c] = x[b, 128s+c, 128k+p]
    xb = np.ascontiguousarray(x[b, :s_len])                       # [S, D]
    xts = xb.reshape(NB, 128, KD, 128).transpose(0, 3, 2, 1)      # [s, p, k, c]
    ins["xt"] = to_bf16(np.ascontiguousarray(xts).reshape(NB, 128, D))
    # this rank's kv-latent shard: contiguous token blocks [4g .. 4g+NB/4)
    nloc = NB // 4
    ins["xkv"] = np.ascontiguousarray(ins["xt"][nloc * g:nloc * (g + 1)])

    # fused q+krope weight: rows = 4x nope(128), 4x qrope(64), krope(64)
    wq3 = wq_w.reshape(H, QK_HD, D)
    rows = [wq3[hg, :NOPE] for hg in heads] + [wq3[hg, NOPE:] for hg in heads]
    rows.append(wkv_a_w[KV:KV + ROPE])                            # krope [64, D]
    wq_sel = np.concatenate(rows, axis=0)                         # [832, D]
    wqt = wq_sel.T.reshape(KD, 128, QW).transpose(1, 0, 2)        # [p, k, 832]
    ins["wqn"] = to_bf16(np.ascontiguousarray(wqt[:, :, 0:512])
                         .reshape(128, KD * 512))
    ins["wqr"] = to_bf16(np.ascontiguousarray(wqt[:, :, 512:QW])
                         .reshape(128, KD * 320))

    wkvt = wkv_a_w[:KV].T.reshape(KD, 128, 512).transpose(1, 0, 2)
    ins["wkv"] = to_bf16(np.ascontiguousarray(wkvt).reshape(128, KD * 512))

    wkv_b3 = wkv_b_w.reshape(H, NOPE + V_HD, KV)
    # wbmt: per head, (wb_h * norm)^T in 4 chunks of [128c, 128d]
    wbt_cols = []
    for hg in heads:
        wb = wkv_b3[hg, :NOPE] * kv_norm_w[None, :]               # [128d, 512c]
        wbt_cols.append(wb.T.reshape(4, 128, 128).transpose(1, 0, 2).reshape(128, 512))
    ins["wbmt"] = to_bf16(np.concatenate(wbt_cols, axis=1))       # [128, 4*512]

    wvt_cols = []
    for hg in heads:
        wv = wkv_b3[hg, NOPE:] * kv_norm_w[None, :]               # [128d, 512c]
        wvt_cols.append(wv.T.reshape(4, 128, 128).transpose(1, 0, 2).reshape(128, 512))
    ins["wvt"] = to_bf16(np.concatenate(wvt_cols, axis=1))        # [128, 4*512]

    wo_cols = np.concatenate([wo_w[:, hg * V_HD:(hg + 1) * V_HD] for hg in heads],
                             axis=1)                              # [D, 512]
    wotl = wo_cols.T.reshape(HL, 128, D).transpose(1, 0, 2)       # [d, h, D]
    ins["wot"] = to_bf16(wotl.reshape(128, HL * D))

    # rope tables in [s-block(128), 64] free-pair layout
    fr = freqs[:s_len]                                            # [S, 32]
    cos2 = np.repeat(np.cos(fr), 2, axis=1).astype(np.float32)    # [S, 64]
    sin1 = np.sin(fr)
    sin2 = np.empty((s_len, ROPE), np.float32)
    sin2[:, 0::2] = -sin1
    sin2[:, 1::2] = sin1
    ins["cosq"] = to_bf16(
        cos2.reshape(NB, 128, 64).transpose(1, 0, 2).reshape(128, NB * 64))
    ins["sinq"] = to_bf16(
        sin2.reshape(NB, 128, 64).transpose(1, 0, 2).reshape(128, NB * 64))

    # transposed diagonal-block 0/1 mask for the S^T scores: [t, q], t > q
    # masked; applied as a post-exp multiply.
    ins["dmask"] = to_bf16(np.where(np.tril(np.ones((128, 128), bool), k=-1),
                                    np.float32(0.0), np.float32(1.0)))
    ins["identb"] = to_bf16(np.eye(128, dtype=np.float32))

    if np.any(wq_b != 0.0):
        rows_b = [wq_b.reshape(H, QK_HD)[hg, :NOPE] for hg in heads] + \
                 [wq_b.reshape(H, QK_HD)[hg, NOPE:] for hg in heads]
        rows_b.append(wkv_a_b[KV:KV + ROPE])
        ins["qb"] = to_bf16(np.concatenate(rows_b)[None, :])
    if np.any(wkv_a_b != 0.0):
        ins["kvb"] = to_bf16(wkv_a_b[:KV][None, :])
    return ins


_nc_cache = {}


def get_nc(s_len, q_bias, kv_bias):
    key = (s_len, q_bias, kv_bias)
    if key not in _nc_cache:
        _nc_cache[key] = build(s_len, q_bias, kv_bias)
    return _nc_cache[key]


def run_cores(inputs, s_len=S, trace=False):
    """Build per-core shards, run the SPMD kernel, return (out, results)."""
    x = np.asarray(inputs["x"], np.float32)
    freqs = np.asarray(inputs["freqs"], np.float32)
    wq_w = np.asarray(inputs["wq_w"], np.float32)
    wq_b = np.asarray(inputs["wq_b"], np.float32)
    wkv_a_w = np.asarray(inputs["wkv_a_w"], np.float32)
    wkv_a_b = np.asarray(inputs["wkv_a_b"], np.float32)
    kv_norm_w = np.asarray(inputs["kv_norm_w"], np.float32)
    wkv_b_w = np.asarray(inputs["wkv_b_w"], np.float32)
    wo_w = np.asarray(inputs["wo_w"], np.float32)
    wo_b = np.asarray(inputs["wo_b"], np.float32)

    q_bias = bool(np.any(wq_b != 0.0) or np.any(wkv_a_b[KV:] != 0.0))
    kv_bias = bool(np.any(wkv_a_b[:KV] != 0.0))
    nc = get_nc(s_len, q_bias, kv_bias)
    in_maps = [
        make_core_inputs(c, x, freqs, wq_w, wq_b, wkv_a_w, wkv_a_b, kv_norm_w,
                         wkv_b_w, wo_w, s_len)
        for c in range(N_CORES)
    ]
    res = bass_utils.run_bass_kernel_spmd(nc, in_maps, core_ids=list(range(N_CORES)),
                                          trace=trace)
    out = np.empty((B, s_len, D), np.float32)
    for b in range(B):
        p = [np.asarray(res.results[TP * b + g]["outT"], np.float32)
                .transpose(1, 0, 2).reshape(D, s_len).T
             for g in range(TP)]
        out[b] = (p[0] + p[1]) + (p[2] + p[3])
    out += wo_b[None, None, :]
    return out, res


def kernel(**inputs) -> np.ndarray:
    out, _ = run_cores(inputs, s_len=S, trace=False)
    return out



# revision 151
# speedup vs baseline: 1.0031x; 1.0005x over previous
"""MLA (DeepSeek-style multi-head latent attention) kernel for Trainium2.

Problem: nn_MultiHeadAttention_28243704939173
  B=2, S=2048, D=2048, H=16, KV_RANK=512, NOPE=128, ROPE=64, V_HD=128.

Sharding (8 NeuronCores): DP=2 over batch x TP=4 over heads (4 heads per
core). The kv-latent projection is sharded over the 4 TP ranks (each rank
projects S/4 contiguous token blocks, fed via the `xkv` input) and the
normalized kv^T shards are exchanged with an in-kernel AllGather over
replica groups [[0..3],[4..7]], overlapped with the q projection. Each
core emits its heads' partial wo projection transposed ([D, S]); the host
sums the 4 TP partials per batch element and adds wo_b.

Structure:
  * Per-head K/V materialization: k_eff_h^T = (wb_h norm) @ kv^T and
    v_eff_h = kv @ (wv_h norm)^T (both 128-d) are small GEMMs; scores then
    contract over 192 dims and PV over 128, instead of both in the 512-d
    latent space (~2.4x less PE work than the absorbed-latent form).
  * Everything runs in bf16 (full PE rate, half the DMA/SBUF, 2x DVE on
    packed copies); PSUM accumulation is fp32. Final rel err ~3e-3 vs the
    2e-2 gate.
  * q-nope is projected DIRECTLY TRANSPOSED (weight chunks stationary, x^T
    moving -> [d, tok] PSUM), so qnt needs no PE transposes; only the rope
    cols (q-rope + k-rope, 320 wide) go token-major for the DVE rope chain
    and get per-block PE transposes.
  * Phase 3 computes scores TRANSPOSED (keys on partitions): the Act exp
    output in SBUF is P^T and feeds the PV matmul directly -- no P
    transposes or PSUM->SBUF P copies. Off-diagonal strip pairs share one
    2-bank PSUM tile / one exp instruction; the causal triangle of the
    diagonal block is a post-exp 0/1 bf16 multiply. The softmax denominator
    is built OFF the PE: a DVE add-tree over the exp strips, a gpsimd
    partition_all_reduce (which also broadcasts), then reciprocal+multiply
    on the oT store. G0's tiny groups interleave with G1's, and wo chunks
    ride inside the attention loop as PE filler (sharing the po PSUM ring),
    so only the last 512-query group's wo runs after the attention drain.
  * The cost model serializes all DMA transfers on one engine pool, so the
    startup issues every front transfer on ONE queue in exact PE-consumption
    order (wkv chunk0, xkb0, remaining wkv, xkb1-3, rope-col weights, rope
    tables, x for q-group 0, nope weights); junk matmuls on a memset tile
    warm the PE pstate inside the initial DMA shadow, and pass-A blocks 0/1
    interleave at half-block granularity to ride the arrival curve. The kv^T
    AllGather (~80us hidden under pass B) feeds phase K, whose psums share
    the phase-3 rings inside one PSUM scope (no boundary stall).
"""
import numpy as np
from contextlib import ExitStack

import ml_dtypes

import concourse.bacc as bacc
import concourse.bass_isa as bass_isa
import concourse.mybir as mybir
import concourse.tile as tile
from concourse import bass_utils

F32 = mybir.dt.float32
BF16 = mybir.dt.bfloat16
AF = mybir.ActivationFunctionType
ALU = mybir.AluOpType

B, S, D = 2, 2048, 2048
H = 16
KV = 512
NOPE, ROPE = 128, 64
QK_HD = NOPE + ROPE
V_HD = 128
SCALE = float(QK_HD) ** -0.5
EPS = 1.1920929e-07
NEG = -1.0e5  # mask addend; NEG*SCALE ~ -7220 -> exp underflows to exactly 0
HL = 4        # local heads per core (TP degree 4)
TP = 4
N_CORES = 8
KD = D // 128   # contraction chunks over the model dim
QW = 832        # fused q-projection width: 4*128 nope + 4*64 qrope + 64 krope


def to_bf16(a: np.ndarray) -> np.ndarray:
    return np.ascontiguousarray(a).astype(ml_dtypes.bfloat16)


def build(s_len: int, q_bias: bool, kv_bias: bool):
    NB = s_len // 128

    nc = bacc.Bacc("TRN2", target_bir_lowering=False, debug=False)

    xt = nc.dram_tensor("xt", [NB, 128, D], BF16, kind="ExternalInput")
    xkv = nc.dram_tensor("xkv", [NB // 4, 128, D], BF16, kind="ExternalInput")
    wqn = nc.dram_tensor("wqn", [128, KD * 512], BF16, kind="ExternalInput")
    wqr = nc.dram_tensor("wqr", [128, KD * 320], BF16, kind="ExternalInput")
    wkv = nc.dram_tensor("wkv", [128, KD * 512], BF16, kind="ExternalInput")
    wbmt = nc.dram_tensor("wbmt", [128, HL * 512], BF16, kind="ExternalInput")
    wvt = nc.dram_tensor("wvt", [128, HL * 512], BF16, kind="ExternalInput")
    wot = nc.dram_tensor("wot", [128, HL * D], BF16, kind="ExternalInput")
    cosq = nc.dram_tensor("cosq", [128, NB * 64], BF16, kind="ExternalInput")
    sinq = nc.dram_tensor("sinq", [128, NB * 64], BF16, kind="ExternalInput")
    dmask = nc.dram_tensor("dmask", [128, 128], BF16, kind="ExternalInput")
    identb = nc.dram_tensor("identb", [128, 128], BF16, kind="ExternalInput")
    if q_bias:
        qb = nc.dram_tensor("qb", [1, QW], BF16, kind="ExternalInput")
    if kv_bias:
        kvb = nc.dram_tensor("kvb", [1, 512], BF16, kind="ExternalInput")
    outT = nc.dram_tensor("outT", [128, D // 128, s_len], BF16,
                          kind="ExternalOutput")

    with tile.TileContext(nc) as tc, ExitStack() as ctx:
        # ---------------- persistent tensors (whole kernel) -----------------
        pe = ctx.enter_context(tc.tile_pool(name="pe", bufs=1))
        identb_sb = pe.tile([128, 128], BF16, tag="identb_sb")
        dmask_sb = pe.tile([128, 128], BF16, tag="dmask_sb")
        qnt = pe.tile([128, HL * s_len], BF16, tag="qnt")       # [d, (h,q)]
        qpet = [pe.tile([128, s_len], BF16, tag=f"qpet{pp}", name=f"qpet{pp}")
                for pp in range(2)]                             # [(2h,r), q]
        kpet = pe.tile([128, s_len], BF16, tag="kpet")          # [r x2, t]
        kvt = pe.tile([128, 4 * s_len], BF16, tag="kvt")        # [c, (cc,t)]
        wbmt_sb = pe.tile([128, HL * 512], BF16, tag="wbmt_sb")
        wvt_sb = pe.tile([128, HL * 512], BF16, tag="wvt_sb")

        # ========== Phase 12: fused kv-latent + q/k-rope projections ========
        with tc.tile_pool(name="p12w", bufs=1) as p12w, \
                tc.tile_pool(name="p12", bufs=3) as p12, \
                tc.tile_pool(name="p12s", bufs=6) as p12s, \
                tc.tile_pool(name="pst", bufs=2, space="PSUM") as pst, \
                tc.tile_pool(name="psq2", bufs=2, space="PSUM") as psq2:
            # Startup DMAs: the cost model serializes all transfers on one
            # DMA-engine resource, so the GLOBAL arrival order must track the
            # PE's consumption order: wkv chunk0 + xkb0 first (pass A block
            # 0), the remaining wkv chunks while block 0 runs, then xkb1-3,
            # then wq groups + xg0 blocks for pass B, misc last. Orders are
            # interleaved across the SP/Act queues (shared HWDGE round-robin).
            NLOC = NB // 4
            xkb = [p12.tile([128, 2048], BF16, tag="xkb", name=f"xkb{ss}",
                            bufs=4) for ss in range(NLOC)]
            wkv_splits = [(0, 1), (1, 1), (2, 2), (4, 2), (6, 2),
                          (8, 2), (10, 2), (12, 2), (14, 2)]
            wkvg = {}
            wkvc = [None] * KD
            for k0, nk_ in wkv_splits:
                wt = p12w.tile([128, 512 * nk_], BF16, tag=f"wkvg{k0}",
                               name=f"wkvg{k0}")
                wkvg[k0] = (wt, nk_)
                for kk in range(nk_):
                    wkvc[k0 + kk] = wt[:, 512 * kk:512 * (kk + 1)]

            def dma_wkv(eng, k0):
                wt, nk_ = wkvg[k0]
                eng.dma_start(wt[:], wkv.ap()[:, 512 * k0:512 * (k0 + nk_)])

            cosq_sb = p12w.tile([128, NB * 64], BF16, tag="cosq_sb")
            sinq_sb = p12w.tile([128, NB * 64], BF16, tag="sinq_sb")
            # q weights split by column type: rope cols are needed first
            # (group-0 ropes run right after pass A), nope cols ~8us later
            wqng = [p12w.tile([128, 2 * 512], BF16, tag="wqng",
                              name=f"wqng{i}", bufs=8) for i in range(8)]
            wqrg = [p12w.tile([128, 2 * 320], BF16, tag="wqrg",
                              name=f"wqrg{i}", bufs=8) for i in range(8)]
            wqnc = [wqng[k // 2][:, 512 * (k % 2):512 * (k % 2 + 1)]
                    for k in range(KD)]
            wqrc = [wqrg[k // 2][:, 320 * (k % 2):320 * (k % 2 + 1)]
                    for k in range(KD)]

            xg0 = p12.tile([128, 4 * 2048], BF16, tag="xg", bufs=3, name="xg0")
            xg0v = xg0[:].rearrange("p (b k c) -> p b k c", b=4, k=KD)

            # PE warm-up: junk matmuls on a memset tile during the initial
            # DMA wait, so the pstate ramp (HAM on real hw) finishes in the
            # shadow and pass A runs at full rate from its first matmul.
            jt = p12w.tile([128, 320], BF16, tag="jt")
            nc.vector.memset(jt[:], 0.0)

            def emit_junk(n):
                for _ in range(n):
                    pj = psq2.tile([128, 320], F32, tag="pq2")
                    nc.tensor.matmul(pj[:], jt[:, 0:128], jt[:],
                                     start=True, stop=True,
                                     skip_group_check=True)

            emit_junk(14)

            # -- the ordered startup sequence: ONE queue (SP), strict demand
            # order, so nothing jumps ahead in the serialized transfer order.
            # (identb rides the Pool SWDGE: 128x128, negligible.)
            dma_wkv(nc.sync, 0)
            nc.sync.dma_start(xkb[0][:, 0:1024], xkv.ap()[0][:, 0:1024])
            nc.sync.dma_start(xkb[0][:, 1024:2048], xkv.ap()[0][:, 1024:2048])
            dma_wkv(nc.sync, 1)
            dma_wkv(nc.sync, 2)
            dma_wkv(nc.sync, 4)
            nc.sync.dma_start(xkb[1][:, 0:1024], xkv.ap()[1][:, 0:1024])
            nc.sync.dma_start(xkb[1][:, 1024:2048], xkv.ap()[1][:, 1024:2048])
            dma_wkv(nc.sync, 6)
            dma_wkv(nc.sync, 8)
            dma_wkv(nc.sync, 10)
            dma_wkv(nc.sync, 12)
            dma_wkv(nc.sync, 14)
            nc.sync.dma_start(xkb[2][:, 0:1024], xkv.ap()[2][:, 0:1024])
            nc.sync.dma_start(xkb[2][:, 1024:2048], xkv.ap()[2][:, 1024:2048])
            nc.sync.dma_start(wqrg[0][:], wqr.ap()[:, 0:640])
            nc.sync.dma_start(xkb[3][:, 0:1024], xkv.ap()[3][:, 0:1024])
            nc.sync.dma_start(xkb[3][:, 1024:2048], xkv.ap()[3][:, 1024:2048])
            for i in range(1, 8):
                nc.sync.dma_start(wqrg[i][:],
                                  wqr.ap()[:, 640 * i:640 * (i + 1)])
            # rope tables: group-0's slice first, the rest off the front
            nc.sync.dma_start(cosq_sb[:, 0:256], cosq.ap()[:, 0:256])
            nc.sync.dma_start(sinq_sb[:, 0:256], sinq.ap()[:, 0:256])
            nc.sync.dma_start(xg0v[:, 0, 0:8, :], xt.ap()[0][:, 0:1024])
            nc.sync.dma_start(xg0v[:, 0, 8:16, :], xt.ap()[0][:, 1024:2048])
            for b in range(1, 4):
                nc.sync.dma_start(xg0v[:, b, 0:8, :], xt.ap()[b][:, 0:1024])
                nc.sync.dma_start(xg0v[:, b, 8:16, :],
                                  xt.ap()[b][:, 1024:2048])
            for i in range(8):
                nc.sync.dma_start(wqng[i][:],
                                  wqn.ap()[:, 1024 * i:1024 * (i + 1)])
            nc.sync.dma_start(cosq_sb[:, 256:NB * 64], cosq.ap()[:, 256:NB * 64])
            nc.sync.dma_start(sinq_sb[:, 256:NB * 64], sinq.ap()[:, 256:NB * 64])
            nc.sync.dma_start(dmask_sb[:], dmask.ap()[:])
            nc.gpsimd.dma_start(identb_sb[:], identb.ap()[:])
            nc.sync.dma_start(wbmt_sb[:], wbmt.ap()[:])
            nc.sync.dma_start(wvt_sb[:], wvt.ap()[:])
            if q_bias or kv_bias:
                ones1 = p12w.tile([1, 128], BF16, tag="ones1")
                nc.vector.memset(ones1[:], 1.0)
            if q_bias:
                qb_sb = p12w.tile([1, QW], BF16, tag="qb_sb")
                nc.scalar.dma_start(qb_sb[:], qb.ap()[:])
                ones512 = p12w.tile([1, 512], BF16, tag="ones512")
                nc.vector.memset(ones512[:], 1.0)
            if kv_bias:
                kvb_sb = p12w.tile([1, 512], BF16, tag="kvb_sb")
                nc.scalar.dma_start(kvb_sb[:], kvb.ap()[:])

            qnt4 = qnt[:].rearrange("p (h n c) -> p h n c", h=HL, n=NB)
            kvt4 = kvt[:].rearrange("p (cc n c) -> p cc n c", cc=4, n=NB)

            # ---- pass A: kv latent for this rank's NB/4 blocks (sharded) ----
            kvsh = p12w.tile([128, 4 * 128 * NLOC], BF16, tag="kvsh")
            kvsh4 = kvsh[:].rearrange("p (cc n c) -> p cc n c", cc=4, n=NLOC)

            def emit_tr_kv(st):
                ss, kv_bf = st
                ptc = pst.tile([128, 512], BF16, tag="ptc")
                for cc in range(4):
                    nc.tensor.transpose(ptc[:, 128 * cc:128 * (cc + 1)],
                                        kv_bf[:, 128 * cc:128 * (cc + 1)],
                                        identb_sb[:])
                nc.vector.tensor_copy(
                    kvsh4[:, :, ss, :],
                    ptc[:].rearrange("p (cc c) -> p cc c", cc=4))

            prevkv = None

            pkvs = {}

            def emit_passA_mm(ss, ka, kb):
                if ka == 0:
                    pkvs[ss] = pskv.tile([128, 512], F32, tag="pkv",
                                         name=f"pkv{ss}")
                pkv = pkvs[ss]
                for k in range(ka, kb):
                    # filler so the warm PE never idles (or resets its
                    # pstate) while the weight-chunk stream catches up
                    if ss == 0 and k in (2, 4, 6, 10, 13):
                        emit_junk(2)
                    lhs = xkb[ss][:, 128 * k:128 * (k + 1)]
                    nc.tensor.matmul(pkv[:], lhs, wkvc[k][:],
                                     start=(k == 0),
                                     stop=(k == KD - 1 and not kv_bias),
                                     skip_group_check=True)
                if kb == KD and kv_bias:
                    nc.tensor.matmul(pkv[:], ones1[:], kvb_sb[:],
                                     start=False, stop=True,
                                     skip_group_check=True)

            def emit_passA(ss):
                nonlocal prevkv
                pkv = pkvs[ss]
                if prevkv is not None:
                    emit_tr_kv(prevkv)
                kvtile = p12.tile([128, 512], F32, tag="kvtile")
                nc.scalar.copy(kvtile[:], pkv[:])
                sq = p12.tile([128, 512], F32, tag="sq", bufs=3)
                msq = p12s.tile([128, 1], F32, tag="msq")
                nc.scalar.activation(sq[:], kvtile[:], AF.Square, bias=0.0,
                                     scale=1.0, accum_out=msq[:])
                ms2 = p12s.tile([128, 1], F32, tag="ms2")
                nc.vector.tensor_scalar(ms2[:], msq[:], 1.0 / KV, EPS, ALU.mult,
                                        ALU.add)
                srt = p12s.tile([128, 1], F32, tag="srt")
                nc.scalar.sqrt(srt[:], ms2[:])
                rrt = p12s.tile([128, 1], F32, tag="rrt")
                nc.vector.reciprocal(rrt[:], srt[:])
                kv_bf = p12.tile([128, 512], BF16, tag="kv_bf", bufs=4)
                nc.vector.tensor_scalar(kv_bf[:], kvtile[:], rrt[:], None,
                                        ALU.mult)
                # krope: bf16 rope chain on this rank's block (sharded)
                prevkv = (ss, kv_bf)

            def emit_collective(ccd, in_b):
                out_b = ccd.tile([TP, 128, 4 * 128 * NLOC], BF16,
                                 name="kv_ag_out")
                nc.gpsimd.dma_start(in_b[:], kvsh[:])
                nc.gpsimd.collective_compute(
                    "AllGather", ALU.bypass,
                    replica_groups=[[0, 1, 2, 3], [4, 5, 6, 7]],
                    ins=[in_b.opt()], outs=[out_b.opt()])
                for r in range(TP):
                    nc.gpsimd.dma_start(
                        kvt4[:, :, NLOC * r:NLOC * (r + 1), :],
                        out_b[r].rearrange("p (cc n c) -> p cc n c",
                                           cc=4, n=NLOC))

            prev = None

            def emit_tr(st):
                """PE transposes for the rope cols of block st (lagged one)."""
                s, qro_bf = st
                ptp = pst.tile([128, 512], BF16, tag="ptc")
                for pp in range(2):
                    nc.tensor.transpose(ptp[:, 128 * pp:128 * (pp + 1)],
                                        qro_bf[:, 128 * pp:128 * (pp + 1)],
                                        identb_sb[:])
                    nc.vector.tensor_copy(qpet[pp][:, 128 * s:128 * (s + 1)],
                                          ptp[:, 128 * pp:128 * (pp + 1)])
                # kpeT duplicated on both partition halves (the rope matmul
                # pairs it with either half of qpet); DVE cannot cross
                # partitions, so transpose twice with an explicit placement.
                nc.tensor.transpose(ptp[0:64, 256:384], qro_bf[:, 256:320],
                                    identb_sb[:], tile_position=(0, 0))
                nc.tensor.transpose(ptp[64:128, 256:384], qro_bf[:, 256:320],
                                    identb_sb[:], tile_position=(0, 64))
                nc.vector.tensor_copy(kpet[:, 128 * s:128 * (s + 1)],
                                      ptp[:, 256:384])

            def emit_rope(s, xg4):
                """Rope q-cols (+krope) for one 128-token block, token-major."""
                nonlocal prev
                b = s % 4
                pq2 = psq2.tile([128, 320], F32, tag="pq2")
                for k in range(KD):
                    nc.tensor.matmul(pq2[:], xg4[:, b, k, :], wqrc[k][:],
                                     start=(k == 0),
                                     stop=(k == KD - 1 and not q_bias),
                                     skip_group_check=True)
                if q_bias:
                    nc.tensor.matmul(pq2[:], ones1[:], qb_sb[:, 512:832],
                                     start=False, stop=True,
                                     skip_group_check=True)
                if prev is not None:
                    emit_tr(prev)
                # whole rope chain in bf16: 2x DVE modes, no final copy (qr
                # stays alive for the lagged transposes, hence bufs=3)
                qr = p12.tile([128, 320], BF16, tag="qr", bufs=4)
                nc.scalar.copy(qr[:], pq2[:])
                qsw = p12.tile([128, 320], BF16, tag="qsw")
                a3 = qr[:].rearrange("p (i two) -> p i two", two=2)
                w3 = qsw[:].rearrange("p (i two) -> p i two", two=2)
                nc.vector.tensor_copy(w3[:, :, 0:1], a3[:, :, 1:2])
                nc.vector.tensor_copy(w3[:, :, 1:2], a3[:, :, 0:1])
                cs = cosq_sb[:, 64 * s:64 * (s + 1)]
                sn = sinq_sb[:, 64 * s:64 * (s + 1)]
                for r in range(5):
                    rsl = qr[:, 64 * r:64 * (r + 1)]
                    ssl = qsw[:, 64 * r:64 * (r + 1)]
                    nc.vector.tensor_mul(rsl, rsl, cs)
                    nc.vector.tensor_mul(ssl, ssl, sn)
                    nc.vector.tensor_add(rsl, rsl, ssl)
                prev = (s, qr)

            def alloc_xg(g):
                """Prefetch the next group's x blocks (ring of 3)."""
                xg = p12.tile([128, 4 * 2048], BF16, tag="xg", bufs=3,
                              name=f"xg{g}")
                xg4 = xg[:].rearrange("p (b k c) -> p b k c", b=4, k=KD)
                for b in range(4):
                    nc.sync.dma_start(xg4[:, b, :, :],
                                      xt.ap()[4 * g + b][:, :])
                return xg4

            def emit_nope(g, xg4, psq1, with_rope):
                """q-nope for one 512-token group: directly transposed
                (weights stationary, x^T moving -> [d, tok] PSUM, no PE
                transposes). Group 0 goes block-by-block with a k-outer loop
                so its x/weight demand is paced with the startup DMA stream;
                later groups (data resident) use group-wide N=512 matmuls."""
                pqts = [psq1.tile([128, 512], F32, tag="pqt",
                                  name=f"pqt{g}_{h}") for h in range(HL)]
                if g == 0:
                    for b in range(4):
                        for k in range(KD):
                            for h in range(HL):
                                nc.tensor.matmul(
                                    pqts[h][:, 128 * b:128 * (b + 1)],
                                    wqnc[k][:, 128 * h:128 * (h + 1)],
                                    xg4[:, b, k, :],
                                    start=(k == 0),
                                    stop=(k == KD - 1 and not q_bias),
                                    skip_group_check=True)
                        if q_bias:
                            for h in range(HL):
                                nc.tensor.matmul(
                                    pqts[h][:, 128 * b:128 * (b + 1)],
                                    qb_sb[:, 128 * h:128 * (h + 1)], ones1[:],
                                    start=False, stop=True,
                                    skip_group_check=True)
                else:
                    for h in range(HL):
                        for k in range(KD):
                            nc.tensor.matmul(
                                pqts[h][:],
                                wqnc[k][:, 128 * h:128 * (h + 1)],
                                xg4[:, :, k, :],
                                start=(k == 0),
                                stop=(k == KD - 1 and not q_bias),
                                skip_group_check=True)
                        if q_bias:
                            nc.tensor.matmul(
                                pqts[h][:],
                                qb_sb[:, 128 * h:128 * (h + 1)], ones512[:],
                                start=False, stop=True, skip_group_check=True)
                        if with_rope:
                            emit_rope(4 * g + h, xg4)
                        # copy as soon as this head's psum completes, so the
                        # next group's pqt ring slot frees ~10us earlier
                        nc.scalar.copy(qnt4[:, h, 4 * g:4 * (g + 1), :]
                                       .rearrange("p n c -> p (n c)"),
                                       pqts[h][:])
                if g == 0:
                    if with_rope:
                        for b in range(4):
                            emit_rope(4 * g + b, xg4)
                    for h in range(HL):
                        nc.scalar.copy(qnt4[:, h, 4 * g:4 * (g + 1), :]
                                       .rearrange("p n c -> p (n c)"),
                                       pqts[h][:])

            # order: pass A; collective; group-0 ropes (fed by the early
            # rope-col weights); group-0 nope; groups 1-3 (nope+rope)
            with tc.tile_pool(name="ccd", bufs=1, space="DRAM") as ccd:
                in_b = ccd.tile([128, 4 * 128 * NLOC], BF16,
                                name="kv_ag_in")
                with tc.tile_pool(name="pskv", bufs=2, space="PSUM") as pskv:
                    # blocks 0/1 interleaved at half-block granularity:
                    # block 1 (fed by the early xkb1 transfer) fills block
                    # 0's weight-arrival stalls
                    emit_passA_mm(0, 0, 8)
                    emit_passA_mm(1, 0, 8)
                    emit_passA_mm(0, 8, KD)
                    emit_passA(0)
                    emit_passA_mm(1, 8, KD)
                    emit_passA(1)
                    for i in range(2, NLOC):
                        emit_passA_mm(i, 0, KD)
                        emit_passA(i)
                    emit_tr_kv(prevkv)
                emit_collective(ccd, in_b)
                for b in range(4):
                    emit_rope(b, xg0v)
                with tc.tile_pool(name="psq1", bufs=4, space="PSUM") as psq1:
                    nxt = alloc_xg(1)
                    emit_nope(0, xg0v, psq1, with_rope=False)
                    for g in range(1, NB // 4):
                        cur = nxt
                        if g < NB // 4 - 1:
                            nxt = alloc_xg(g + 1)
                        emit_nope(g, cur, psq1, with_rope=True)
                    emit_tr(prev)

        # ---------------- persistent tensors (post-P12) ---------------------
        pl = ctx.enter_context(tc.tile_pool(name="pl", bufs=1))
        keff = pl.tile([128, HL * s_len], BF16, tag="keff")   # [d, (h,t)]
        veff = pl.tile([128, HL * s_len], BF16, tag="veff")   # [t, (h,j*d)]
        ot_bf = pl.tile([128, HL * s_len], BF16, tag="ot_bf")  # [d, (h,q)]
        wot_sb = pl.tile([128, HL * D], BF16, tag="wot_sb")
        nc.scalar.dma_start(wot_sb[:], wot.ap()[:])

        # ========== Phase K: materialize per-head K/V =======================
        NG = s_len // 512

        # ========== Phases K+3 (one PSUM scope, no boundary stall) =========
        # Scores are computed transposed (keys on partitions, queries on the
        # free dim), so the Act exp output in SBUF IS P^T and feeds the PV
        # matmul directly -- no transposes, no PSUM->SBUF P copies. The
        # softmax denominator is built OFF the PE: a DVE add-tree over the
        # exp strips followed by a gpsimd partition_all_reduce (which also
        # broadcasts), then a single DVE divide on the oT store. Off-diagonal
        # strips are paired into 2-bank PSUM tiles so one exp instruction
        # covers both. The causal mask inside the diagonal 128x128 block is a
        # post-exp multiply by a 0/1 bf16 triangle (exp(x-1e5*m) == exp(x)*t).
        with tc.tile_pool(name="expp", bufs=1) as expp, \
                tc.tile_pool(name="accp", bufs=4) as accp, \
                tc.tile_pool(name="rzp", bufs=4) as rzp, \
                tc.tile_pool(name="pssp", bufs=2, space="PSUM") as pssp, \
                tc.tile_pool(name="pssd", bufs=2, space="PSUM") as pssd, \
                tc.tile_pool(name="ps3o", bufs=2, space="PSUM") as ps3o:
            # ---- phase K: per-head K/V materialization (psums share the
            # pssd ring so the K->3 transition reuses warm banks) ----
            for h in range(HL):
                for tg in range(NG):
                    pk = pssd.tile([128, 512], F32, tag="psd")
                    for cc in range(4):
                        nc.tensor.matmul(
                            pk[:],
                            wbmt_sb[:, 512 * h + 128 * cc:512 * h + 128 * (cc + 1)],
                            kvt[:, s_len * cc + 512 * tg:s_len * cc + 512 * (tg + 1)],
                            start=(cc == 0), stop=(cc == 3))
                    nc.scalar.copy(
                        keff[:, s_len * h + 512 * tg:s_len * h + 512 * (tg + 1)],
                        pk[:])
            # veff: one [t, (h,d)] psum per 128-token block -- N=512 matmuls
            # across all 4 heads at once (4x fewer instructions than per-head)
            wvt4 = wvt_sb[:].rearrange("p (h c) -> p h c", h=HL)
            veff3 = veff[:].rearrange("p (h t) -> p h t", h=HL)
            for j in range(NB):
                pv = ps3o.tile([128, 512], F32, tag="po")
                for cc in range(4):
                    nc.tensor.matmul(
                        pv[:],
                        kvt[:, s_len * cc + 128 * j:s_len * cc + 128 * (j + 1)],
                        wvt4[:, :, 128 * cc:128 * (cc + 1)],
                        start=(cc == 0), stop=(cc == 3),
                        skip_group_check=True)
                nc.vector.tensor_copy(
                    veff3[:, :, 128 * j:128 * (j + 1)],
                    pv[:].rearrange("p (h c) -> p h c", h=HL))

            def emit_scores(h, G):
                """Transposed scores + exp; off-diag strips paired per PSUM."""
                strips = []
                hb = 64 * (h % 2)
                q0 = s_len * h + 512 * G
                k0 = s_len * h

                def score_mms(dst, j, c0):
                    nc.tensor.matmul(
                        dst,
                        keff[:, k0 + 128 * j:k0 + 128 * (j + 1)],
                        qnt[:, q0 + 128 * c0:q0 + 512],
                        start=True, stop=False, skip_group_check=True)
                    nc.tensor.matmul(
                        dst,
                        kpet[hb:hb + 64, 128 * j:128 * (j + 1)],
                        qpet[h // 2][hb:hb + 64,
                                     512 * G + 128 * c0:512 * (G + 1)],
                        start=False, stop=True, skip_group_check=True)

                for jp in range(2 * G):
                    ps = pssp.tile([128, 1024], F32, tag="psp")
                    for u in range(2):
                        score_mms(ps[:, 512 * u:512 * (u + 1)], 2 * jp + u, 0)
                    et = expp.tile([128, 1024], BF16, tag="etp", bufs=15)
                    nc.scalar.activation(et[:], ps[:], AF.Exp, bias=0.0,
                                         scale=SCALE)
                    strips.append(("pair", jp, et))
                for ii in range(4):
                    j = 4 * G + ii
                    ps = pssd.tile([128, 512], F32, tag="psd")
                    score_mms(ps[:, 128 * ii:512], j, ii)
                    et = expp.tile([128, 512], BF16, tag="etd", bufs=11)
                    nc.scalar.activation(et[:, 128 * ii:512], ps[:, 128 * ii:512],
                                         AF.Exp, bias=0.0, scale=SCALE)
                    # zero the masked (t > q) triangle of the diagonal block
                    nc.vector.tensor_mul(et[:, 128 * ii:128 * (ii + 1)],
                                         et[:, 128 * ii:128 * (ii + 1)],
                                         dmask_sb[:])
                    strips.append(("diag", ii, et))
                return strips

            def emit_pv(h, G, strips):
                """PV accumulation + off-PE softmax denominator, /Z on store."""
                po = ps3o.tile([128, 512], F32, tag="po")
                acc = accp.tile([128, 512], BF16, tag="acc")
                last = 4 * G + 3
                k0 = s_len * h
                nacc = 0
                for kind, idx, et in strips:
                    if kind == "pair":
                        for u in range(2):
                            j = 2 * idx + u
                            nc.tensor.matmul(
                                po[:],
                                veff[:, k0 + 128 * j:k0 + 128 * (j + 1)],
                                et[:, 512 * u:512 * (u + 1)],
                                start=(j == 0), stop=(j == last),
                                skip_group_check=True)
                        if idx == 0:
                            nc.vector.tensor_add(acc[:], et[:, 0:512],
                                                 et[:, 512:1024])
                        else:
                            nc.vector.tensor_add(acc[:], acc[:], et[:, 0:512])
                            nc.vector.tensor_add(acc[:], acc[:], et[:, 512:1024])
                        nacc += 2
                    else:
                        ii = idx
                        j = 4 * G + ii
                        nc.tensor.matmul(
                            po[:, 128 * ii:512],
                            veff[:, k0 + 128 * j:k0 + 128 * (j + 1)],
                            et[:, 128 * ii:512],
                            start=(j == 0), stop=(j == last),
                            skip_group_check=True)
                        if nacc == 0:
                            nc.vector.tensor_copy(acc[:], et[:, 0:512])
                        else:
                            nc.vector.tensor_add(acc[:, 128 * ii:512],
                                                 acc[:, 128 * ii:512],
                                                 et[:, 128 * ii:512])
                        nacc += 1
                zr = rzp.tile([128, 512], F32, tag="zr")
                nc.gpsimd.partition_all_reduce(zr[:], acc[:], 128,
                                               bass_isa.ReduceOp.add)
                rz = rzp.tile([128, 512], F32, tag="rz")
                nc.vector.reciprocal(rz[:], zr[:])
                nc.vector.tensor_mul(
                    ot_bf[:, s_len * h + 512 * G:s_len * h + 512 * (G + 1)],
                    po[:], rz[:])

            # ---- phase 4 chunks (wo projection), interleaved into the
            # attention loop as PE filler: wo(G-1, ng) rides between the
            # score groups of G, sharing the "po" PSUM ring ----
            def emit_wo_chunk(G, ng, p4, tail=False):
                last_chunk = (G == NG - 1 and ng == D // 512 - 1)
                osb = p4.tile([128, 2048], BF16, tag="osb")
                for nn in range(4):
                    n = 4 * ng + nn
                    # after attention drains, the diag-score ring is idle --
                    # alternating psums halves the ring-handoff waits
                    pool_ = pssd if tail and nn % 2 else ps3o
                    pw = pool_.tile([128, 512], F32,
                                    tag="psd" if tail and nn % 2 else "po")
                    for h in range(HL):
                        nc.tensor.matmul(
                            pw[:],
                            wot_sb[:, D * h + 128 * n:D * h + 128 * (n + 1)],
                            ot_bf[:, s_len * h + 512 * G:s_len * h + 512 * (G + 1)],
                            start=(h == 0), stop=(h == HL - 1))
                    nc.scalar.copy(osb[:, 512 * nn:512 * (nn + 1)], pw[:])
                    if last_chunk:
                        eng = nc.sync if nn % 2 == 0 else nc.scalar
                        eng.dma_start(
                            outT.ap()[:, 4 * ng + nn, 512 * G:512 * (G + 1)],
                            osb[:, 512 * nn:512 * (nn + 1)])
                if not last_chunk:
                    nc.sync.dma_start(
                        outT.ap()[:, 4 * ng:4 * (ng + 1),
                                  512 * G:512 * (G + 1)],
                        osb[:].rearrange("p (n q) -> p n q", n=4))

            # G0's tiny groups interleave with G1's so the exp latency of the
            # short strips hides behind bigger PE work; wo chunks ride along
            # once their group's last divide is in flight.
            work = [(h // 2, G) for h in range(2 * HL) for G in (0, 1)
                    if (h % 2 == 0) == (G == 0)]
            work += [(h, 2) for h in range(HL)] + [(h, 3) for h in range(HL)]
            wo_after = {(h, 2): [(0, h)] for h in range(HL)}
            wo_after.update({(h, 3): [(1, h), (2, h)] for h in range(HL)})
            with tc.tile_pool(name="p4", bufs=4) as p4:
                pending = None
                for h, G in work:
                    strips = emit_scores(h, G)
                    if pending is not None:
                        emit_pv(*pending)
                    pending = (h, G, strips)
                    for woG, wong in wo_after.get((h, G), []):
                        emit_wo_chunk(woG, wong, p4)
                emit_pv(*pending)
                for ng in range(D // 512):
                    emit_wo_chunk(NG - 1, ng, p4, tail=True)

    nc.compile()
    return nc


def make_core_inputs(core, x, freqs, wq_w, wq_b, wkv_a_w, wkv_a_b, kv_norm_w,
                     wkv_b_w, wo_w, s_len):
    """Host-side shard + layout prep for one core."""
    b, g = core // TP, core % TP
    NB = s_len // 128
    heads = [TP * g + hh for hh in range(HL)]  # heads for TP rank g

    ins = {}
    # xt[s, p, 128k+c] = x[b, 128s+c, 128k+p]
    xb = np.ascontiguousarray(x[b, :s_len])                       # [S, D]
    xts = xb.reshape(NB, 128, KD, 128).transpose(0, 3, 2, 1)      # [s, p, k, c]
    ins["xt"] = to_bf16(np.ascontiguousarray(xts).reshape(NB, 128, D))
    # this rank's kv-latent shard: contiguous token blocks [4g .. 4g+NB/4)
    nloc = NB // 4
    ins["xkv"] = np.ascontiguousarray(ins["xt"][nloc * g:nloc * (g + 1)])

    # fused q+krope weight: rows = 4x nope(128), 4x qrope(64), krope(64)
    wq3 = wq_w.reshape(H, QK_HD, D)
    rows = [wq3[hg, :NOPE] for hg in heads] + [wq3[hg, NOPE:] for hg in heads]
    rows.append(wkv_a_w[KV:KV + ROPE])                            # krope [64, D]
    wq_sel = np.concatenate(rows, axis=0)                         # [832, D]
    wqt = wq_sel.T.reshape(KD, 128, QW).transpose(1, 0, 2)        # [p, k, 832]
    ins["wqn"] = to_bf16(np.ascontiguousarray(wqt[:, :, 0:512])
                         .reshape(128, KD * 512))
    ins["wqr"] = to_bf16(np.ascontiguousarray(wqt[:, :, 512:QW])
                         .reshape(128, KD * 320))

    wkvt = wkv_a_w[:KV].T.reshape(KD, 128, 512).transpose(1, 0, 2)
    ins["wkv"] = to_bf16(np.ascontiguousarray(wkvt).reshape(128, KD * 512))

    wkv_b3 = wkv_b_w.reshape(H, NOPE + V_HD, KV)
    # wbmt: per head, (wb_h * norm)^T in 4 chunks of [128c, 128d]
    wbt_cols = []
    for hg in heads:
        wb = wkv_b3[hg, :NOPE] * kv_norm_w[None, :]               # [128d, 512c]
        wbt_cols.append(wb.T.reshape(4, 128, 128).transpose(1, 0, 2).reshape(128, 512))
    ins["wbmt"] = to_bf16(np.concatenate(wbt_cols, axis=1))       # [128, 4*512]

    wvt_cols = []
    for hg in heads:
        wv = wkv_b3[hg, NOPE:] * kv_norm_w[None, :]               # [128d, 512c]
        wvt_cols.append(wv.T.reshape(4, 128, 128).transpose(1, 0, 2).reshape(128, 512))
    ins["wvt"] = to_bf16(np.concatenate(wvt_cols, axis=1))        # [128, 4*512]

    wo_cols = np.concatenate([wo_w[:, hg * V_HD:(hg + 1) * V_HD] for hg in heads],
                             axis=1)                              # [D, 512]
    wotl = wo_cols.T.reshape(HL, 128, D).transpose(1, 0, 2)       # [d, h, D]
    ins["wot"] = to_bf16(wotl.reshape(128, HL * D))

    # rope tables in [s-block(128), 64] free-pair layout
    fr = freqs[:s_len]                                            # [S, 32]
    cos2 = np.repeat(np.cos(fr), 2, axis=1).astype(np.float32)    # [S, 64]
    sin1 = np.sin(fr)
    sin2 = np.empty((s_len, ROPE), np.float32)
    sin2[:, 0::2] = -sin1
    sin2[:, 1::2] = sin1
    ins["cosq"] = to_bf16(
        cos2.reshape(NB, 128, 64).transpose(1, 0, 2).reshape(128, NB * 64))
    ins["sinq"] = to_bf16(
        sin2.reshape(NB, 128, 64).transpose(1, 0, 2).reshape(128, NB * 64))

    # transposed diagonal-block 0/1 mask for the S^T scores: [t, q], t > q
    # masked; applied as a post-exp multiply.
    ins["dmask"] = to_bf16(np.where(np.tril(np.ones((128, 128), bool), k=-1),
                                    np.float32(0.0), np.float32(1.0)))
    ins["identb"] = to_bf16(np.eye(128, dtype=np.float32))

    if np.any(wq_b != 0.0):
        rows_b = [wq_b.reshape(H, QK_HD)[hg, :NOPE] for hg in heads] + \
                 [wq_b.reshape(H, QK_HD)[hg, NOPE:] for hg in heads]
        rows_b.append(wkv_a_b[KV:KV + ROPE])
        ins["qb"] = to_bf16(np.concatenate(rows_b)[None, :])
    if np.any(wkv_a_b != 0.0):
        ins["kvb"] = to_bf16(wkv_a_b[:KV][None, :])
    return ins


_nc_cache = {}


def get_nc(s_len, q_bias, kv_bias):
    key = (s_len, q_bias, kv_bias)
    if key not in _nc_cache:
        _nc_cache[key] = build(s_len, q_bias, kv_bias)
    return _nc_cache[key]


def run_cores(inputs, s_len=S, trace=False):
    """Build per-core shards, run the SPMD kernel, return (out, results)."""
    x = np.asarray(inputs["x"], np.float32)
    freqs = np.asarray(inputs["freqs"], np.float32)
    wq_w = np.asarray(inputs["wq_w"], np.float32)
    wq_b = np.asarray(inputs["wq_b"], np.float32)
    wkv_a_w = np.asarray(inputs["wkv_a_w"], np.float32)
    wkv_a_b = np.asarray(inputs["wkv_a_b"], np.float32)
    kv_norm_w = np.asarray(inputs["kv_norm_w"], np.float32)
    wkv_b_w = np.asarray(inputs["wkv_b_w"], np.float32)
    wo_w = np.asarray(inputs["wo_w"], np.float32)
    wo_b = np.asarray(inputs["wo_b"], np.float32)

    q_bias = bool(np.any(wq_b != 0.0) or np.any(wkv_a_b[KV:] != 0.0))
    kv_bias = bool(np.any(wkv_a_b[:KV] != 0.0))
    nc = get_nc(s_len, q_bias, kv_bias)
    in_maps = [
        make_core_inputs(c, x, freqs, wq_w, wq_b, wkv_a_w, wkv_a_b, kv_norm_w,
                         wkv_b_w, wo_w, s_len)
        for c in range(N_CORES)
    ]
    res = bass_utils.run_bass_kernel_spmd(nc, in_maps, core_ids=list(range(N_CORES)),
                                          trace=trace)
    out = np.empty((B, s_len, D), np.float32)
    for b in range(B):
        p = [np.asarray(res.results[TP * b + g]["outT"], np.float32)
                .transpose(1, 0, 2).reshape(D, s_len).T
             for g in range(TP)]
        out[b] = (p[0] + p[1]) + (p[2] + p[3])
    out += wo_b[None, None, :]
    return out, res


def kernel(**inputs) -> np.ndarray:
    out, _ = run_cores(inputs, s_len=S, trace=False)
    return out

